# revision 41
# baseline (speedup 1.0000x reference)
"""Causal self-attention (B=4, T=2048, C=1024, H=16) on 8 TRN2 NeuronCores.

Sharding: core c -> batch b = c//2, head-group g2 = c%2 (8 heads, feature
columns j0 = g2*512 .. +512).  Each core:
  - QKV projections for its 512-wide slice (Megatron column-parallel),
  - causal attention for its 8 heads (softmax without max-subtraction:
    logits ~ N(0,1), folded 1/sqrt(hd) into Wq on host),
  - partial output projection y_half @ Wp[:, slice].T.
Host sums the two partials per batch.  No collectives.

v3 dataflow (per core):
  - QKV projections in fp8e4 DoubleRow (2 k-tiles / matmul, 0.5 cyc/row)
    with first-order error compensation: x and W are split hi/lo into two
    fp8 tensors on the host (per-tensor power-of-2 scales keep the lo parts
    out of the fp8 subnormal range); psum accumulates xh*wh + xh*wl + xl*wh.
  - QK^T per k-tile in fp32r (exp amplifies logit error; fp8 not safe).
  - exp on ScalarE with scale=2^-13 (undoes the host scales), bf16 out.
  - causal staircase zeroed by gpsimd affine_select (bf16).
  - AV with the attention tile as the *stationary* operand [128k x 128q]
    and [v | ones]-moving (65 columns out): rowsum lands in psum column 64,
    so normalization is a per-partition reciprocal + one broadcast multiply
    on DVE (no broadcast matmuls).  AV psums accumulate with start=False
    onto gpsimd-memset banks (8 interleaved groups share 2 banks; the HW
    2KB zero-region would corrupt interleaved start=True groups).
  - y transposed back to [i, t] via PE transpose (bf16, 2 heads / transpose),
    then the output projection contracts in bf16.
  - fused software-pipelined schedule: the attention phase is ACT(exp)-
    limited, so next-t-block projections and previous-block transposes/
    out-projections are drained into the attention blocks as background PE
    work between k-tiles (the `Bg` queue of emission generators).
"""
import numpy as np

B, T, C = 4, 2048, 1024
NC = 8
P = 128
CO = 8           # c-tiles of 128 (contraction for QKV)
NCP = 4          # co-pairs (DoubleRow contracts 2 c-tiles per matmul)
QB = 512         # t_q block
NQB = T // QB    # 4
NKT = T // P     # 16 k-tiles
D = 64           # head dim
W65 = 65         # [v | ones]

S_Q = 256.0      # host scale on Wq/8 (fp8 dynamic range)
S_K = 32.0       # host scale on Wk
S_V = 32.0       # host scale on Wv (cancels in normalization via ones=S_V)
EXP_SCALE = 1.0 / (S_Q * S_K)   # 2^-13, applied inside the exp activation

_CACHE = {}

# build-time tuning knobs (sweepable)
CFG = {"attp": 2, "aep": 12, "lag": 7, "quantum": 832}


class Bg:
    """Queue of emission generators drained between attention k-tiles.

    Each generator yields the matmul-row count it just emitted; items are
    labeled so attention blocks can force-drain their dependencies.
    """

    def __init__(self):
        self.items = []

    def add(self, label, gen):
        self.items.append((label, gen))

    def rows_left(self):
        return sum(1 for _ in self.items)  # item count proxy (unused)

    def drain_rows(self, target):
        done = 0
        while self.items and done < target:
            _, g = self.items[0]
            try:
                done += next(g)
            except StopIteration:
                self.items.pop(0)
        return done

    def drain_until(self, label):
        while any(l == label for l, _ in self.items):
            _, g = self.items[0]
            try:
                next(g)
            except StopIteration:
                self.items.pop(0)

    def drain_all(self):
        while self.items:
            _, g = self.items[0]
            try:
                next(g)
            except StopIteration:
                self.items.pop(0)


def _build():
    from contextlib import ExitStack
    import concourse.bass as bass
    import concourse.tile as tile
    from concourse import bacc, mybir

    F32 = mybir.dt.float32
    F32R = mybir.dt.float32r
    F8 = mybir.dt.float8e4
    BF16 = mybir.dt.bfloat16
    AF = mybir.ActivationFunctionType
    MUL = mybir.AluOpType.mult
    DR = mybir.MatmulPerfMode.DoubleRow

    nc = bacc.Bacc("TRN2", target_bir_lowering=False, debug=False,
                   dynamic_dma_scratch_size=2048)
    xh = nc.dram_tensor("xh", [C, T], F8, kind="ExternalInput").ap()
    xl = nc.dram_tensor("xl", [C, T], F8, kind="ExternalInput").ap()
    wts_d = {}
    for nm in ("wqh", "wql", "wkh", "wkl", "wvh", "wvl"):
        wts_d[nm] = nc.dram_tensor(nm, [C, 512], F8, kind="ExternalInput").ap()
    wp = nc.dram_tensor("wp", [512, C], BF16, kind="ExternalInput").ap()
    on1 = nc.dram_tensor("on1", [P, CO], BF16, kind="ExternalInput").ap()
    idt = nc.dram_tensor("idt", [P, P], BF16, kind="ExternalInput").ap()
    out = nc.dram_tensor("out", [T, C], F32, kind="ExternalOutput").ap()

    xh3 = xh.rearrange("(co ci) t -> ci co t", ci=P)     # [128, 8, 2048]
    xl3 = xl.rearrange("(co ci) t -> ci co t", ci=P)
    w3 = {nm: a.rearrange("(co ci) j -> ci co j", ci=P)
          for nm, a in wts_d.items()}                    # [128, 8, 512]
    wp3 = wp.rearrange("(go gi) m -> gi go m", gi=P)     # [128, 4, 1024]

    with tile.TileContext(nc) as tc, ExitStack() as ctx:
        persist = ctx.enter_context(tc.tile_pool(name="persist", bufs=1))
        qt = [persist.tile([P, T], F32R, tag=f"qt{g}", name=f"qt{g}") for g in range(4)]
        kt = [persist.tile([P, T], F32R, tag=f"kt{g}", name=f"kt{g}") for g in range(4)]
        vtp = persist.tile([P, NKT, CO, W65], BF16, tag="vtp", name="vtp")
        # normalized y, qtile-major: [q-pos, qtile, h2, d] (contiguous
        # [128,128] per-qtile slice for the PE transpose)
        ynm = [persist.tile([P, NKT, 2, D], BF16, tag=f"ynm{g}", name=f"ynm{g}")
               for g in range(4)]
        yts = [persist.tile([P, T], BF16, tag=f"yts{g}", name=f"yts{g}")
               for g in range(4)]
        on1t = persist.tile([P, CO], BF16, tag="on1", name="on1")
        idtt = persist.tile([P, P], BF16, tag="idt", name="idt")
        wpt = persist.tile([P, 4, C], BF16, tag="wpt", name="wpt")

        xtp = ctx.enter_context(tc.tile_pool(name="xtp", bufs=2))
        wpool = ctx.enter_context(tc.tile_pool(name="wqkv", bufs=1))
        bgp = ctx.enter_context(
            tc.tile_pool(name="bgp", bufs=2, space="PSUM"))
        attp = ctx.enter_context(
            tc.tile_pool(name="attp", bufs=CFG["attp"], space="PSUM"))
        avp = ctx.enter_context(tc.tile_pool(name="avp", bufs=1, space="PSUM"))
        aep = ctx.enter_context(tc.tile_pool(name="aep", bufs=CFG["aep"]))
        rcp = ctx.enter_context(tc.tile_pool(name="rcp", bufs=2))
        outp = ctx.enter_context(tc.tile_pool(name="outp", bufs=4))

        wt = {}
        for nm in ("wqh", "wql", "wkh", "wkl", "wvh", "wvl"):
            wt[nm] = wpool.tile([P, CO, 512], F8, tag=nm, name=nm)

        # ---- input DMAs: hi parts on sync, lo parts on vector (parallel
        # queues halve the head's arrival ramp); wk on gpsimd, v/wp on scalar
        xts = {}
        xh0 = xtp.tile([P, CO, QB], F8, tag="xh", name="xh0")
        xl0 = xtp.tile([P, CO, QB], F8, tag="xl", name="xl0")
        xts[0] = (xh0, xl0)
        for cp in range(NCP):
            s = slice(2 * cp, 2 * cp + 2)
            nc.sync.dma_start(wt["wqh"][:, s], w3["wqh"][:, s])
            nc.sync.dma_start(xh0[:, s], xh3[:, s, 0:QB])
            nc.gpsimd.dma_start(wt["wql"][:, s], w3["wql"][:, s])
            nc.gpsimd.dma_start(xl0[:, s], xl3[:, s, 0:QB])
        nc.scalar.dma_start(wt["wkh"][:], w3["wkh"])
        nc.scalar.dma_start(wt["wkl"][:], w3["wkl"])
        nc.scalar.dma_start(on1t[:], on1)
        nc.scalar.dma_start(idtt[:], idt)
        nc.scalar.dma_start(wt["wvh"][:], w3["wvh"])
        nc.scalar.dma_start(wt["wvl"][:], w3["wvl"])
        nc.scalar.dma_start(wpt[:], wp3)
        # rowsum column of v: ones * S_V (gpsimd; DVE is busy with psum moves)
        nc.gpsimd.tensor_copy(
            vtp[:, :, :, D:W65],
            on1t[:, None, :, None].broadcast_to((P, NKT, CO, 1)))

        # ---------- emission generators ----------
        def gen_qk(proj, g, tb):
            """q/k projection for one 128-wide j-slice, one 512-t block."""
            wh, wl = wt[f"w{proj}h"], wt[f"w{proj}l"]
            xh_t, xl_t = xts[tb]
            terms = ((xh_t, wh), (xh_t, wl), (xl_t, wh))
            dst = (qt if proj == "q" else kt)[g]
            gs = slice(g * P, (g + 1) * P)
            ps = bgp.tile([P, QB], F32, tag="bg", name=f"{proj}{g}t{tb}")
            # halves sequential: a start=True re-arms the bank zero-region
            for h in range(2):
                hs = slice(h * 256, (h + 1) * 256)
                for cp in range(NCP):
                    s = slice(2 * cp, 2 * cp + 2)
                    for ti, (mv, st) in enumerate(terms):
                        nc.tensor.matmul(
                            ps[:, hs], st[:, s, gs], mv[:, s, hs],
                            start=(cp == 0 and ti == 0),
                            stop=(cp == NCP - 1 and ti == 2),
                            perf_mode=DR)
                        yield 128
            nc.vector.tensor_copy(dst[:, tb * QB:(tb + 1) * QB], ps[:])
            yield 0

        def gen_v(tt, tb):
            """v projection (natural layout) for one 128-t tile."""
            wh, wl = wt["wvh"], wt["wvl"]
            xh_t, xl_t = xts[tb]
            terms = ((xh_t, wh), (xh_t, wl), (xl_t, wh))
            ki = tb * 4 + tt
            ts_ = slice(tt * P, (tt + 1) * P)
            ps = bgp.tile([P, QB], F32, tag="bg", name=f"v{ki}")
            for h in range(2):
                hs = slice(h * 256, (h + 1) * 256)
                for cp in range(NCP):
                    s = slice(2 * cp, 2 * cp + 2)
                    for ti, (mv, st) in enumerate(terms):
                        nc.tensor.matmul(
                            ps[:, hs], mv[:, s, ts_], st[:, s, hs],
                            start=(cp == 0 and ti == 0),
                            stop=(cp == NCP - 1 and ti == 2),
                            perf_mode=DR)
                        yield 128
            nc.vector.tensor_copy(
                vtp[:, ki, :, 0:D],
                ps[:].rearrange("p (h d) -> p h d", d=D))
            yield 0

        def gen_tr(g, qb):
            """transpose y_norm -> yT for one head-pair, one 512-t block."""
            psf = bgp.tile([P, QB], F32, tag="bg", name=f"tr{g}q{qb}")
            tp = psf[:].bitcast(BF16)[:, 0:QB].rearrange(
                "p (a b) -> p a b", a=4)
            for qt_ in range(4):
                nc.tensor.matmul(
                    tp[:, qt_, :],
                    ynm[g][:, qb * 4 + qt_, :, :].rearrange("p a b -> p (a b)"),
                    idtt[:], is_transpose=True)
                yield 128
            nc.vector.tensor_copy(
                yts[g][:, qb * QB:(qb + 1) * QB].rearrange(
                    "p (a b) -> p a b", a=4),
                tp[:])
            yield 0

        def gen_po(tt, mh, copy_on_scalar=False):
            """output projection for one [128 t, 512 m] tile + store."""
            po = bgp.tile([P, QB], F32, tag="bg", name=f"po{tt}m{mh}")
            for g in range(4):
                nc.tensor.matmul(
                    po[:], yts[g][:, tt * P:(tt + 1) * P],
                    wpt[:, g, mh * QB:(mh + 1) * QB],
                    start=(g == 0), stop=(g == 3))
                yield 512
            ob = outp.tile([P, QB], F32, tag="ob", name="ob")
            if copy_on_scalar:
                nc.scalar.activation(ob[:], po[:], AF.Copy)
            else:
                nc.vector.tensor_copy(ob[:], po[:])
            nc.sync.dma_start(
                out[tt * P:(tt + 1) * P, mh * QB:(mh + 1) * QB], ob[:])
            yield 0

        def run(gen):
            for _ in gen:
                pass

        # ---------- attention block ----------
        def att_block(g, qb, bg, pre_av=None):
            q0 = qb * QB
            ks = list(range(qb * 4, qb * 4 + 4)) + list(range(0, qb * 4))
            yp = avp.tile([P, 4, 2, P], F32, tag="yp", name="yp")
            nc.vector.memset(yp[:, :, :, 0:W65], 0.0)
            barrier = [pre_av]

            def av(job):
                if barrier[0] is not None:
                    barrier[0]()
                    barrier[0] = None
                ki, d, ae = job
                qt0 = 0 if d < 0 else d // P
                for h2 in range(2):
                    h = 2 * g + h2
                    for qt_ in range(qt0, 4):
                        nc.tensor.matmul(
                            yp[:, qt_, h2, 0:W65],
                            ae[:, h2, qt_ * P:(qt_ + 1) * P],
                            vtp[:, ki, h, 0:W65],
                            start=False, stop=True, skip_group_check=True)

            pend = []
            for idx, ki in enumerate(ks):
                d = (ki - qb * 4) * P if ki >= qb * 4 else -1
                dq = d if d in (P, 2 * P) else (2 * P if d == 3 * P else 0)
                ap_t = attp.tile([P, 2, QB], F32, tag="att", name="att")
                for h2 in range(2):
                    rows = slice(h2 * D, h2 * D + D)
                    nc.tensor.matmul(
                        ap_t[:, h2, dq:QB],
                        kt[g][rows, ki * P:(ki + 1) * P],
                        qt[g][rows, q0 + dq:q0 + QB],
                        start=True, stop=True)
                ae = aep.tile([P, 2, QB], BF16, tag="ae", name="ae")
                e0 = max(d, 0)
                nc.scalar.activation(ae[:, :, e0:QB], ap_t[:, :, e0:QB],
                                     AF.Exp, scale=EXP_SCALE)
                if d >= 0:
                    for h2 in range(2):
                        nc.gpsimd.affine_select(
                            out=ae[:, h2, d:d + P],
                            in_=ae[:, h2, d:d + P],
                            compare_op=mybir.AluOpType.is_ge,
                            fill=0.0, base=0,
                            pattern=[[1, P]], channel_multiplier=-1)
                pend.append((ki, d, ae))
                if len(pend) > CFG["lag"]:
                    av(pend.pop(0))
                bg.drain_rows(CFG["quantum"])
            while pend:
                av(pend.pop(0))

            rc = rcp.tile([P, 4, 2], F32, tag="rc", name="rc")
            nc.vector.reciprocal_approx_fast(rc[:], yp[:, :, :, D])
            nc.vector.tensor_tensor(
                ynm[g][:, qb * 4:(qb + 1) * 4, :, :],
                yp[:, :, :, 0:D],
                rc[:, :, :, None].broadcast_to((P, 4, 2, D)), MUL)

        # ---------- fused schedule ----------
        bg = Bg()
        # head: only q/k for head-pair 0; v follows in bg (first-AV barrier)
        run(gen_qk("q", 0, 0))
        run(gen_qk("k", 0, 0))
        for tt in range(4):
            bg.add("v0", gen_v(tt, 0))
        for g in range(1, 4):
            bg.add(f"qk{g}t0", gen_qk("q", g, 0))
            bg.add(f"qk{g}t0", gen_qk("k", g, 0))

        for qb in range(NQB):
            tbn = qb + 1
            if tbn < NQB:
                xh_t = xtp.tile([P, CO, QB], F8, tag="xh", name=f"xh{tbn}")
                xl_t = xtp.tile([P, CO, QB], F8, tag="xl", name=f"xl{tbn}")
                nc.sync.dma_start(xh_t[:], xh3[:, :, tbn * QB:(tbn + 1) * QB])
                nc.gpsimd.dma_start(xl_t[:], xl3[:, :, tbn * QB:(tbn + 1) * QB])
                xts[tbn] = (xh_t, xl_t)
                for tt in range(4):
                    bg.add(f"v{tbn}", gen_v(tt, tbn))
                for g in range(2):
                    bg.add(f"qk{g}t{tbn}", gen_qk("q", g, tbn))
                    bg.add(f"qk{g}t{tbn}", gen_qk("k", g, tbn))
            if qb > 0:
                for g in range(4):
                    bg.add(f"tr{qb - 1}", gen_tr(g, qb - 1))
                for tt in range(4 * (qb - 1), 4 * qb):
                    for mh in range(2):
                        bg.add(f"po{qb - 1}", gen_po(tt, mh))
            if tbn < NQB:
                # late j-slices feed the back half of this qb / early next qb
                for g in range(2, 4):
                    bg.add(f"qk{g}t{tbn}", gen_qk("q", g, tbn))
                    bg.add(f"qk{g}t{tbn}", gen_qk("k", g, tbn))
            for g in range(4):
                bg.drain_until(f"qk{g}t{qb}")
                pre = (lambda q_=qb: bg.drain_until(f"v{q_}")) if g == 0 else None
                att_block(g, qb, bg, pre_av=pre)
                if qb == NQB - 1:
                    bg.add("tr3", gen_tr(g, qb))

        bg.drain_all()
        for tt in range(4 * (NQB - 1), 4 * NQB):
            for mh in range(2):
                run(gen_po(tt, mh, copy_on_scalar=True))

    nc.finalize()
    return nc


def _prep_inputs(x, Wq, Wk, Wv, Wp):
    import ml_dtypes
    F8 = ml_dtypes.float8_e4m3
    BF = ml_dtypes.bfloat16
    f32 = np.float32

    def hilo(a):
        hi = np.ascontiguousarray(a).astype(F8)
        lo = (a - hi.astype(f32)).astype(F8)
        return hi, lo

    on1 = np.full((P, CO), S_V, BF)
    idt = np.eye(P, dtype=BF)
    in_maps = []
    for c in range(NC):
        b, g2 = c // 2, c % 2
        j0 = g2 * 512
        xhc, xlc = hilo(x[b].T.astype(f32))
        wqh, wql = hilo((Wq[j0:j0 + 512] * (S_Q / 8.0)).T.astype(f32))
        wkh, wkl = hilo((Wk[j0:j0 + 512] * S_K).T.astype(f32))
        wvh, wvl = hilo((Wv[j0:j0 + 512] * S_V).T.astype(f32))
        in_maps.append({
            "xh": xhc, "xl": xlc,
            "wqh": wqh, "wql": wql,
            "wkh": wkh, "wkl": wkl,
            "wvh": wvh, "wvl": wvl,
            "wp": np.ascontiguousarray(Wp[:, j0:j0 + 512].T).astype(BF),
            "on1": on1, "idt": idt,
        })
    return in_maps


def kernel(x, Wq, Wk, Wv, Wp, _trace=False):
    from concourse.bass_utils import run_bass_kernel_spmd

    x = np.asarray(x); Wq = np.asarray(Wq); Wk = np.asarray(Wk)
    Wv = np.asarray(Wv); Wp = np.asarray(Wp)

    if "nc" not in _CACHE:
        _CACHE["nc"] = _build()
    nc = _CACHE["nc"]

    in_maps = _prep_inputs(x, Wq, Wk, Wv, Wp)
    res = run_bass_kernel_spmd(nc, in_maps, core_ids=list(range(NC)),
                               trace=_trace)
    outs = [r["out"] for r in res.results]
    full = np.empty((B, T, C), np.float32)
    for b in range(B):
        full[b] = outs[2 * b] + outs[2 * b + 1]
    if _trace:
        _CACHE["last_results"] = res
    return full


# revision 42
# speedup vs baseline: 1.0074x; 1.0074x over previous
"""Causal self-attention (B=4, T=2048, C=1024, H=16) on 8 TRN2 NeuronCores.

Sharding: core c -> batch b = c//2, head-group g2 = c%2 (8 heads, feature
columns j0 = g2*512 .. +512).  Each core:
  - QKV projections for its 512-wide slice (Megatron column-parallel),
  - causal attention for its 8 heads (softmax without max-subtraction:
    logits ~ N(0,1), folded 1/sqrt(hd) into Wq on host),
  - partial output projection y_half @ Wp[:, slice].T.
Host sums the two partials per batch.  No collectives.

v3 dataflow (per core):
  - QKV projections in fp8e4 DoubleRow (2 k-tiles / matmul, 0.5 cyc/row)
    with first-order error compensation: x and W are split hi/lo into two
    fp8 tensors on the host (per-tensor power-of-2 scales keep the lo parts
    out of the fp8 subnormal range); psum accumulates xh*wh + xh*wl + xl*wh.
  - QK^T per k-tile in fp32r (exp amplifies logit error; fp8 not safe).
  - exp on ScalarE with scale=2^-13 (undoes the host scales), bf16 out.
  - causal staircase zeroed by gpsimd affine_select (bf16).
  - AV with the attention tile as the *stationary* operand [128k x 128q]
    and [v | ones]-moving (65 columns out): rowsum lands in psum column 64,
    so normalization is a per-partition reciprocal + one broadcast multiply
    on DVE (no broadcast matmuls).  AV psums accumulate with start=False
    onto gpsimd-memset banks (8 interleaved groups share 2 banks; the HW
    2KB zero-region would corrupt interleaved start=True groups).
  - y transposed back to [i, t] via PE transpose (bf16, 2 heads / transpose),
    then the output projection contracts in bf16.
  - fused software-pipelined schedule: the attention phase is ACT(exp)-
    limited, so next-t-block projections and previous-block transposes/
    out-projections are drained into the attention blocks as background PE
    work between k-tiles (the `Bg` queue of emission generators).
"""
import numpy as np

B, T, C = 4, 2048, 1024
NC = 8
P = 128
CO = 8           # c-tiles of 128 (contraction for QKV)
NCP = 4          # co-pairs (DoubleRow contracts 2 c-tiles per matmul)
QB = 512         # t_q block
NQB = T // QB    # 4
NKT = T // P     # 16 k-tiles
D = 64           # head dim
W65 = 65         # [v | ones]

S_Q = 256.0      # host scale on Wq/8 (fp8 dynamic range)
S_K = 32.0       # host scale on Wk
S_V = 32.0       # host scale on Wv (cancels in normalization via ones=S_V)
EXP_SCALE = 1.0 / (S_Q * S_K)   # 2^-13, applied inside the exp activation

_CACHE = {}

# build-time tuning knobs (sweepable)
CFG = {"attp": 2, "aep": 12, "lag": 7, "quantum": 832}


class Bg:
    """Queue of emission generators drained between attention k-tiles.

    Each generator yields the matmul-row count it just emitted; items are
    labeled so attention blocks can force-drain their dependencies.
    """

    def __init__(self):
        self.items = []

    def add(self, label, gen):
        self.items.append((label, gen))

    def rows_left(self):
        return sum(1 for _ in self.items)  # item count proxy (unused)

    def drain_rows(self, target):
        done = 0
        while self.items and done < target:
            _, g = self.items[0]
            try:
                done += next(g)
            except StopIteration:
                self.items.pop(0)
        return done

    def drain_until(self, label):
        while any(l == label for l, _ in self.items):
            _, g = self.items[0]
            try:
                next(g)
            except StopIteration:
                self.items.pop(0)

    def drain_all(self):
        while self.items:
            _, g = self.items[0]
            try:
                next(g)
            except StopIteration:
                self.items.pop(0)


def _build():
    from contextlib import ExitStack
    import concourse.bass as bass
    import concourse.tile as tile
    from concourse import bacc, mybir

    F32 = mybir.dt.float32
    F32R = mybir.dt.float32r
    F8 = mybir.dt.float8e4
    BF16 = mybir.dt.bfloat16
    AF = mybir.ActivationFunctionType
    MUL = mybir.AluOpType.mult
    DR = mybir.MatmulPerfMode.DoubleRow

    nc = bacc.Bacc("TRN2", target_bir_lowering=False, debug=False,
                   dynamic_dma_scratch_size=2048)
    xh = nc.dram_tensor("xh", [C, T], F8, kind="ExternalInput").ap()
    xl = nc.dram_tensor("xl", [C, T], F8, kind="ExternalInput").ap()
    wts_d = {}
    for nm in ("wqh", "wql", "wkh", "wkl", "wvh", "wvl"):
        wts_d[nm] = nc.dram_tensor(nm, [C, 512], F8, kind="ExternalInput").ap()
    wp = nc.dram_tensor("wp", [512, C], BF16, kind="ExternalInput").ap()
    on1 = nc.dram_tensor("on1", [P, CO], BF16, kind="ExternalInput").ap()
    idt = nc.dram_tensor("idt", [P, P], BF16, kind="ExternalInput").ap()
    out = nc.dram_tensor("out", [T, C], F32, kind="ExternalOutput").ap()

    xh3 = xh.rearrange("(co ci) t -> ci co t", ci=P)     # [128, 8, 2048]
    xl3 = xl.rearrange("(co ci) t -> ci co t", ci=P)
    w3 = {nm: a.rearrange("(co ci) j -> ci co j", ci=P)
          for nm, a in wts_d.items()}                    # [128, 8, 512]
    wp3 = wp.rearrange("(go gi) m -> gi go m", gi=P)     # [128, 4, 1024]

    with tile.TileContext(nc) as tc, ExitStack() as ctx:
        persist = ctx.enter_context(tc.tile_pool(name="persist", bufs=1))
        qt = [persist.tile([P, T], F32R, tag=f"qt{g}", name=f"qt{g}") for g in range(4)]
        kt = [persist.tile([P, T], F32R, tag=f"kt{g}", name=f"kt{g}") for g in range(4)]
        vtp = persist.tile([P, NKT, CO, W65], BF16, tag="vtp", name="vtp")
        # normalized y, qtile-major: [q-pos, qtile, h2, d] (contiguous
        # [128,128] per-qtile slice for the PE transpose)
        ynm = [persist.tile([P, NKT, 2, D], BF16, tag=f"ynm{g}", name=f"ynm{g}")
               for g in range(4)]
        yts = [persist.tile([P, T], BF16, tag=f"yts{g}", name=f"yts{g}")
               for g in range(4)]
        on1t = persist.tile([P, CO], BF16, tag="on1", name="on1")
        idtt = persist.tile([P, P], BF16, tag="idt", name="idt")
        wpt = persist.tile([P, 4, C], BF16, tag="wpt", name="wpt")

        xtp = ctx.enter_context(tc.tile_pool(name="xtp", bufs=2))
        wpool = ctx.enter_context(tc.tile_pool(name="wqkv", bufs=1))
        bgp = ctx.enter_context(
            tc.tile_pool(name="bgp", bufs=2, space="PSUM"))
        attp = ctx.enter_context(
            tc.tile_pool(name="attp", bufs=CFG["attp"], space="PSUM"))
        avp = ctx.enter_context(tc.tile_pool(name="avp", bufs=1, space="PSUM"))
        aep = ctx.enter_context(tc.tile_pool(name="aep", bufs=CFG["aep"]))
        rcp = ctx.enter_context(tc.tile_pool(name="rcp", bufs=2))
        outp = ctx.enter_context(tc.tile_pool(name="outp", bufs=4))

        wt = {}
        for nm in ("wqh", "wql", "wkh", "wkl", "wvh", "wvl"):
            wt[nm] = wpool.tile([P, CO, 512], F8, tag=nm, name=nm)

        # ---- input DMAs: hi parts on sync, lo parts on vector (parallel
        # queues halve the head's arrival ramp); wk on gpsimd, v/wp on scalar
        xts = {}
        xh0 = xtp.tile([P, CO, QB], F8, tag="xh", name="xh0")
        xl0 = xtp.tile([P, CO, QB], F8, tag="xl", name="xl0")
        xts[0] = (xh0, xl0)
        for cp in range(NCP):
            s = slice(2 * cp, 2 * cp + 2)
            nc.sync.dma_start(wt["wqh"][:, s], w3["wqh"][:, s])
            nc.sync.dma_start(xh0[:, s], xh3[:, s, 0:QB])
            nc.gpsimd.dma_start(wt["wql"][:, s], w3["wql"][:, s])
            nc.gpsimd.dma_start(xl0[:, s], xl3[:, s, 0:QB])
        nc.scalar.dma_start(wt["wkh"][:], w3["wkh"])
        nc.scalar.dma_start(wt["wkl"][:], w3["wkl"])
        nc.scalar.dma_start(on1t[:], on1)
        nc.scalar.dma_start(idtt[:], idt)
        nc.scalar.dma_start(wt["wvh"][:], w3["wvh"])
        nc.scalar.dma_start(wt["wvl"][:], w3["wvl"])
        nc.scalar.dma_start(wpt[:], wp3)
        # rowsum column of v: ones * S_V (gpsimd; DVE is busy with psum moves)
        nc.gpsimd.tensor_copy(
            vtp[:, :, :, D:W65],
            on1t[:, None, :, None].broadcast_to((P, NKT, CO, 1)))

        # ---------- emission generators ----------
        def gen_qk(proj, g, tb):
            """q/k projection for one 128-wide j-slice, one 512-t block."""
            wh, wl = wt[f"w{proj}h"], wt[f"w{proj}l"]
            xh_t, xl_t = xts[tb]
            terms = ((xh_t, wh), (xh_t, wl), (xl_t, wh))
            dst = (qt if proj == "q" else kt)[g]
            gs = slice(g * P, (g + 1) * P)
            ps = bgp.tile([P, QB], F32, tag="bg", name=f"{proj}{g}t{tb}")
            # halves sequential: a start=True re-arms the bank zero-region
            for h in range(2):
                hs = slice(h * 256, (h + 1) * 256)
                for cp in range(NCP):
                    s = slice(2 * cp, 2 * cp + 2)
                    for ti, (mv, st) in enumerate(terms):
                        nc.tensor.matmul(
                            ps[:, hs], st[:, s, gs], mv[:, s, hs],
                            start=(cp == 0 and ti == 0),
                            stop=(cp == NCP - 1 and ti == 2),
                            perf_mode=DR)
                        yield 128
            nc.vector.tensor_copy(dst[:, tb * QB:(tb + 1) * QB], ps[:])
            yield 0

        def gen_v(tt, tb):
            """v projection (natural layout) for one 128-t tile."""
            wh, wl = wt["wvh"], wt["wvl"]
            xh_t, xl_t = xts[tb]
            terms = ((xh_t, wh), (xh_t, wl), (xl_t, wh))
            ki = tb * 4 + tt
            ts_ = slice(tt * P, (tt + 1) * P)
            ps = bgp.tile([P, QB], F32, tag="bg", name=f"v{ki}")
            for h in range(2):
                hs = slice(h * 256, (h + 1) * 256)
                for cp in range(NCP):
                    s = slice(2 * cp, 2 * cp + 2)
                    for ti, (mv, st) in enumerate(terms):
                        nc.tensor.matmul(
                            ps[:, hs], mv[:, s, ts_], st[:, s, hs],
                            start=(cp == 0 and ti == 0),
                            stop=(cp == NCP - 1 and ti == 2),
                            perf_mode=DR)
                        yield 128
            nc.vector.tensor_copy(
                vtp[:, ki, :, 0:D],
                ps[:].rearrange("p (h d) -> p h d", d=D))
            yield 0

        def gen_tr(g, qb):
            """transpose y_norm -> yT for one head-pair, one 512-t block."""
            psf = bgp.tile([P, QB], F32, tag="bg", name=f"tr{g}q{qb}")
            tp = psf[:].bitcast(BF16)[:, 0:QB].rearrange(
                "p (a b) -> p a b", a=4)
            for qt_ in range(4):
                nc.tensor.matmul(
                    tp[:, qt_, :],
                    ynm[g][:, qb * 4 + qt_, :, :].rearrange("p a b -> p (a b)"),
                    idtt[:], is_transpose=True)
                yield 128
            nc.vector.tensor_copy(
                yts[g][:, qb * QB:(qb + 1) * QB].rearrange(
                    "p (a b) -> p a b", a=4),
                tp[:])
            yield 0

        def gen_po(tt, mh, copy_on_scalar=False):
            """output projection for one [128 t, 512 m] tile + store."""
            po = bgp.tile([P, QB], F32, tag="bg", name=f"po{tt}m{mh}")
            for g in range(4):
                nc.tensor.matmul(
                    po[:], yts[g][:, tt * P:(tt + 1) * P],
                    wpt[:, g, mh * QB:(mh + 1) * QB],
                    start=(g == 0), stop=(g == 3))
                yield 512
            ob = outp.tile([P, QB], F32, tag="ob", name="ob")
            if copy_on_scalar:
                nc.scalar.activation(ob[:], po[:], AF.Copy)
            else:
                nc.vector.tensor_copy(ob[:], po[:])
            nc.sync.dma_start(
                out[tt * P:(tt + 1) * P, mh * QB:(mh + 1) * QB], ob[:])
            yield 0

        def run(gen):
            for _ in gen:
                pass

        # ---------- attention block ----------
        def att_block(g, qb, bg, pre_av=None):
            q0 = qb * QB
            ks = list(range(qb * 4, qb * 4 + 4)) + list(range(0, qb * 4))
            yp = avp.tile([P, 4, 2, P], F32, tag="yp", name="yp")
            nc.vector.memset(yp[:, :, :, 0:W65], 0.0)
            barrier = [pre_av]

            def av(job):
                if barrier[0] is not None:
                    barrier[0]()
                    barrier[0] = None
                ki, d, ae = job
                qt0 = 0 if d < 0 else d // P
                for h2 in range(2):
                    h = 2 * g + h2
                    for qt_ in range(qt0, 4):
                        nc.tensor.matmul(
                            yp[:, qt_, h2, 0:W65],
                            ae[:, h2, qt_ * P:(qt_ + 1) * P],
                            vtp[:, ki, h, 0:W65],
                            start=False, stop=True, skip_group_check=True)

            pend = []
            for idx, ki in enumerate(ks):
                d = (ki - qb * 4) * P if ki >= qb * 4 else -1
                dq = d if d in (P, 2 * P) else (2 * P if d == 3 * P else 0)
                ap_t = attp.tile([P, 2, QB], F32, tag="att", name="att")
                for h2 in range(2):
                    rows = slice(h2 * D, h2 * D + D)
                    nc.tensor.matmul(
                        ap_t[:, h2, dq:QB],
                        kt[g][rows, ki * P:(ki + 1) * P],
                        qt[g][rows, q0 + dq:q0 + QB],
                        start=True, stop=True)
                ae = aep.tile([P, 2, QB], BF16, tag="ae", name="ae")
                e0 = max(d, 0)
                nc.scalar.activation(ae[:, :, e0:QB], ap_t[:, :, e0:QB],
                                     AF.Exp, scale=EXP_SCALE)
                if d >= 0:
                    for h2 in range(2):
                        nc.gpsimd.affine_select(
                            out=ae[:, h2, d:d + P],
                            in_=ae[:, h2, d:d + P],
                            compare_op=mybir.AluOpType.is_ge,
                            fill=0.0, base=0,
                            pattern=[[1, P]], channel_multiplier=-1)
                pend.append((ki, d, ae))
                if len(pend) > CFG["lag"]:
                    av(pend.pop(0))
                bg.drain_rows(CFG["quantum"])
            while pend:
                av(pend.pop(0))

            rc = rcp.tile([P, 4, 2], F32, tag="rc", name="rc")
            nc.vector.reciprocal_approx_fast(rc[:], yp[:, :, :, D])
            nc.vector.tensor_tensor(
                ynm[g][:, qb * 4:(qb + 1) * 4, :, :],
                yp[:, :, :, 0:D],
                rc[:, :, :, None].broadcast_to((P, 4, 2, D)), MUL)

        # ---------- fused schedule ----------
        bg = Bg()
        # head: only q/k for head-pair 0; v follows in bg (first-AV barrier)
        run(gen_qk("q", 0, 0))
        run(gen_qk("k", 0, 0))
        for tt in range(4):
            bg.add("v0", gen_v(tt, 0))
        for g in range(1, 4):
            bg.add(f"qk{g}t0", gen_qk("q", g, 0))
            bg.add(f"qk{g}t0", gen_qk("k", g, 0))

        for qb in range(NQB):
            tbn = qb + 1
            if tbn < NQB:
                xh_t = xtp.tile([P, CO, QB], F8, tag="xh", name=f"xh{tbn}")
                xl_t = xtp.tile([P, CO, QB], F8, tag="xl", name=f"xl{tbn}")
                nc.sync.dma_start(xh_t[:], xh3[:, :, tbn * QB:(tbn + 1) * QB])
                nc.gpsimd.dma_start(xl_t[:], xl3[:, :, tbn * QB:(tbn + 1) * QB])
                xts[tbn] = (xh_t, xl_t)
                for tt in range(4):
                    bg.add(f"v{tbn}", gen_v(tt, tbn))
                for g in range(2):
                    bg.add(f"qk{g}t{tbn}", gen_qk("q", g, tbn))
                    bg.add(f"qk{g}t{tbn}", gen_qk("k", g, tbn))
            if qb > 0:
                for g in range(4):
                    bg.add(f"tr{qb - 1}", gen_tr(g, qb - 1))
            if qb >= 2:
                for tt in range(4 * (qb - 2), 4 * (qb - 1)):
                    for mh in range(2):
                        bg.add(f"po{qb - 2}", gen_po(tt, mh))
            if qb == NQB - 1:
                # last qb gets extra filler: its predecessor's out-proj
                for tt in range(4 * (qb - 1), 4 * qb):
                    for mh in range(2):
                        bg.add(f"po{qb - 1}", gen_po(tt, mh))
            if tbn < NQB:
                # late j-slices feed the back half of this qb / early next qb
                for g in range(2, 4):
                    bg.add(f"qk{g}t{tbn}", gen_qk("q", g, tbn))
                    bg.add(f"qk{g}t{tbn}", gen_qk("k", g, tbn))
            for g in range(4):
                bg.drain_until(f"qk{g}t{qb}")
                pre = (lambda q_=qb: bg.drain_until(f"v{q_}")) if g == 0 else None
                att_block(g, qb, bg, pre_av=pre)
                if qb == NQB - 1:
                    bg.add("tr3", gen_tr(g, qb))

        bg.drain_all()
        for tt in range(4 * (NQB - 1), 4 * NQB):
            for mh in range(2):
                run(gen_po(tt, mh, copy_on_scalar=True))

    nc.finalize()
    return nc


def _prep_inputs(x, Wq, Wk, Wv, Wp):
    import ml_dtypes
    F8 = ml_dtypes.float8_e4m3
    BF = ml_dtypes.bfloat16
    f32 = np.float32

    def hilo(a):
        hi = np.ascontiguousarray(a).astype(F8)
        lo = (a - hi.astype(f32)).astype(F8)
        return hi, lo

    on1 = np.full((P, CO), S_V, BF)
    idt = np.eye(P, dtype=BF)
    in_maps = []
    for c in range(NC):
        b, g2 = c // 2, c % 2
        j0 = g2 * 512
        xhc, xlc = hilo(x[b].T.astype(f32))
        wqh, wql = hilo((Wq[j0:j0 + 512] * (S_Q / 8.0)).T.astype(f32))
        wkh, wkl = hilo((Wk[j0:j0 + 512] * S_K).T.astype(f32))
        wvh, wvl = hilo((Wv[j0:j0 + 512] * S_V).T.astype(f32))
        in_maps.append({
            "xh": xhc, "xl": xlc,
            "wqh": wqh, "wql": wql,
            "wkh": wkh, "wkl": wkl,
            "wvh": wvh, "wvl": wvl,
            "wp": np.ascontiguousarray(Wp[:, j0:j0 + 512].T).astype(BF),
            "on1": on1, "idt": idt,
        })
    return in_maps


def kernel(x, Wq, Wk, Wv, Wp, _trace=False):
    from concourse.bass_utils import run_bass_kernel_spmd

    x = np.asarray(x); Wq = np.asarray(Wq); Wk = np.asarray(Wk)
    Wv = np.asarray(Wv); Wp = np.asarray(Wp)

    if "nc" not in _CACHE:
        _CACHE["nc"] = _build()
    nc = _CACHE["nc"]

    in_maps = _prep_inputs(x, Wq, Wk, Wv, Wp)
    res = run_bass_kernel_spmd(nc, in_maps, core_ids=list(range(NC)),
                               trace=_trace)
    outs = [r["out"] for r in res.results]
    full = np.empty((B, T, C), np.float32)
    for b in range(B):
        full[b] = outs[2 * b] + outs[2 * b + 1]
    if _trace:
        _CACHE["last_results"] = res
    return full


# revision 46
# speedup vs baseline: 1.0135x; 1.0061x over previous
"""Causal self-attention (B=4, T=2048, C=1024, H=16) on 8 TRN2 NeuronCores.

Sharding: core c -> batch b = c//2, head-group g2 = c%2 (8 heads, feature
columns j0 = g2*512 .. +512).  Each core:
  - QKV projections for its 512-wide slice (Megatron column-parallel),
  - causal attention for its 8 heads (softmax without max-subtraction:
    logits ~ N(0,1), folded 1/sqrt(hd) into Wq on host),
  - partial output projection y_half @ Wp[:, slice].T.
Host sums the two partials per batch.  No collectives.

v3 dataflow (per core):
  - QKV projections in fp8e4 DoubleRow (2 k-tiles / matmul, 0.5 cyc/row)
    with first-order error compensation: x and W are split hi/lo into two
    fp8 tensors on the host (per-tensor power-of-2 scales keep the lo parts
    out of the fp8 subnormal range); psum accumulates xh*wh + xh*wl + xl*wh.
  - QK^T per k-tile in fp32r (exp amplifies logit error; fp8 not safe).
  - exp on ScalarE with scale=2^-13 (undoes the host scales), bf16 out.
  - causal staircase zeroed by gpsimd affine_select (bf16).
  - AV with the attention tile as the *stationary* operand [128k x 128q]
    and [v | ones]-moving (65 columns out): rowsum lands in psum column 64,
    so normalization is a per-partition reciprocal + one broadcast multiply
    on DVE (no broadcast matmuls).  AV psums accumulate with start=False
    onto gpsimd-memset banks (8 interleaved groups share 2 banks; the HW
    2KB zero-region would corrupt interleaved start=True groups).
  - y transposed back to [i, t] via PE transpose (bf16, 2 heads / transpose),
    then the output projection contracts in bf16.
  - fused software-pipelined schedule: the attention phase is ACT(exp)-
    limited, so next-t-block projections and previous-block transposes/
    out-projections are drained into the attention blocks as background PE
    work between k-tiles (the `Bg` queue of emission generators).
"""
import numpy as np

B, T, C = 4, 2048, 1024
NC = 8
P = 128
CO = 8           # c-tiles of 128 (contraction for QKV)
NCP = 4          # co-pairs (DoubleRow contracts 2 c-tiles per matmul)
QB = 512         # t_q block
NQB = T // QB    # 4
NKT = T // P     # 16 k-tiles
D = 64           # head dim
W65 = 65         # [v | ones]

S_Q = 256.0      # host scale on Wq/8 (fp8 dynamic range)
S_K = 32.0       # host scale on Wk
S_V = 32.0       # host scale on Wv
Y_S = 8.0        # y_norm scale: ones = S_V/Y_S makes y_norm = Y_S * y
S_P = 32.0       # host scale on Wp (fp8 dynamic range)
EXP_SCALE = 1.0 / (S_Q * S_K)   # 2^-13, applied inside the exp activation
OUT_SCALE = 1.0 / (Y_S * S_P)   # folded into the final psum->sbuf copy

_CACHE = {}

# build-time tuning knobs (sweepable)
CFG = {"attp": 2, "aep": 12, "lag": 7, "quantum": 832}


class Bg:
    """Queue of emission generators drained between attention k-tiles.

    Each generator yields the matmul-row count it just emitted; items are
    labeled so attention blocks can force-drain their dependencies.
    """

    def __init__(self):
        self.items = []

    def add(self, label, gen):
        self.items.append((label, gen))

    def rows_left(self):
        return sum(1 for _ in self.items)  # item count proxy (unused)

    def drain_rows(self, target):
        done = 0
        while self.items and done < target:
            _, g = self.items[0]
            try:
                done += next(g)
            except StopIteration:
                self.items.pop(0)
        return done

    def drain_until(self, label):
        while any(l == label for l, _ in self.items):
            _, g = self.items[0]
            try:
                next(g)
            except StopIteration:
                self.items.pop(0)

    def drain_all(self):
        while self.items:
            _, g = self.items[0]
            try:
                next(g)
            except StopIteration:
                self.items.pop(0)


def _build():
    from contextlib import ExitStack
    import concourse.bass as bass
    import concourse.tile as tile
    from concourse import bacc, mybir

    F32 = mybir.dt.float32
    F32R = mybir.dt.float32r
    F8 = mybir.dt.float8e4
    BF16 = mybir.dt.bfloat16
    AF = mybir.ActivationFunctionType
    MUL = mybir.AluOpType.mult
    DR = mybir.MatmulPerfMode.DoubleRow

    nc = bacc.Bacc("TRN2", target_bir_lowering=False, debug=False,
                   dynamic_dma_scratch_size=2048)
    xh = nc.dram_tensor("xh", [C, T], F8, kind="ExternalInput").ap()
    xl = nc.dram_tensor("xl", [C, T], F8, kind="ExternalInput").ap()
    wts_d = {}
    for nm in ("wqh", "wql", "wkh", "wkl", "wvh", "wvl"):
        wts_d[nm] = nc.dram_tensor(nm, [C, 512], F8, kind="ExternalInput").ap()
    wph = nc.dram_tensor("wph", [512, C], F8, kind="ExternalInput").ap()
    wpl = nc.dram_tensor("wpl", [512, C], F8, kind="ExternalInput").ap()
    on1 = nc.dram_tensor("on1", [P, CO], BF16, kind="ExternalInput").ap()
    idt = nc.dram_tensor("idt", [P, P], BF16, kind="ExternalInput").ap()
    out = nc.dram_tensor("out", [T, C], F32, kind="ExternalOutput").ap()

    xh3 = xh.rearrange("(co ci) t -> ci co t", ci=P)     # [128, 8, 2048]
    xl3 = xl.rearrange("(co ci) t -> ci co t", ci=P)
    w3 = {nm: a.rearrange("(co ci) j -> ci co j", ci=P)
          for nm, a in wts_d.items()}                    # [128, 8, 512]
    wph3 = wph.rearrange("(go gi) m -> gi go m", gi=P)   # [128, 4, 1024]
    wpl3 = wpl.rearrange("(go gi) m -> gi go m", gi=P)

    with tile.TileContext(nc) as tc, ExitStack() as ctx:
        persist = ctx.enter_context(tc.tile_pool(name="persist", bufs=1))
        qt = [persist.tile([P, T], F32R, tag=f"qt{g}", name=f"qt{g}") for g in range(4)]
        kt = [persist.tile([P, T], F32R, tag=f"kt{g}", name=f"kt{g}") for g in range(4)]
        vtp = persist.tile([P, NKT, CO, W65], BF16, tag="vtp", name="vtp")
        # normalized y, qtile-major: [q-pos, qtile, h2, d] (contiguous
        # [128,128] per-qtile slice for the PE transpose)
        ynm = [persist.tile([P, NKT, 2, D], BF16, tag=f"ynm{g}", name=f"ynm{g}")
               for g in range(4)]
        yts = [persist.tile([P, T], BF16, tag=f"yts{g}", name=f"yts{g}")
               for g in range(4)]
        ytsh = persist.tile([P, 4, T], F8, tag="ytsh", name="ytsh")
        ytsl = persist.tile([P, 4, T], F8, tag="ytsl", name="ytsl")
        on1t = persist.tile([P, CO], BF16, tag="on1", name="on1")
        idtt = persist.tile([P, P], BF16, tag="idt", name="idt")
        wpth = persist.tile([P, 4, C], F8, tag="wpth", name="wpth")
        wptl = persist.tile([P, 4, C], F8, tag="wptl", name="wptl")

        xtp = ctx.enter_context(tc.tile_pool(name="xtp", bufs=2))
        wpool = ctx.enter_context(tc.tile_pool(name="wqkv", bufs=1))
        bgp = ctx.enter_context(
            tc.tile_pool(name="bgp", bufs=2, space="PSUM"))
        attp = ctx.enter_context(
            tc.tile_pool(name="attp", bufs=CFG["attp"], space="PSUM"))
        avp = ctx.enter_context(tc.tile_pool(name="avp", bufs=1, space="PSUM"))
        aep = ctx.enter_context(tc.tile_pool(name="aep", bufs=CFG["aep"]))
        rcp = ctx.enter_context(tc.tile_pool(name="rcp", bufs=2))
        outp = ctx.enter_context(tc.tile_pool(name="outp", bufs=4))

        wt = {}
        for nm in ("wqh", "wql", "wkh", "wkl", "wvh", "wvl"):
            wt[nm] = wpool.tile([P, CO, 512], F8, tag=nm, name=nm)

        # ---- input DMAs: hi parts on sync, lo parts on vector (parallel
        # queues halve the head's arrival ramp); wk on gpsimd, v/wp on scalar
        xts = {}
        xh0 = xtp.tile([P, CO, QB], F8, tag="xh", name="xh0")
        xl0 = xtp.tile([P, CO, QB], F8, tag="xl", name="xl0")
        xts[0] = (xh0, xl0)
        for cp in range(NCP):
            s = slice(2 * cp, 2 * cp + 2)
            nc.sync.dma_start(wt["wqh"][:, s], w3["wqh"][:, s])
            nc.sync.dma_start(xh0[:, s], xh3[:, s, 0:QB])
            nc.gpsimd.dma_start(wt["wql"][:, s], w3["wql"][:, s])
            nc.gpsimd.dma_start(xl0[:, s], xl3[:, s, 0:QB])
        nc.scalar.dma_start(wt["wkh"][:], w3["wkh"])
        nc.scalar.dma_start(wt["wkl"][:], w3["wkl"])
        nc.scalar.dma_start(on1t[:], on1)
        nc.scalar.dma_start(idtt[:], idt)
        nc.scalar.dma_start(wt["wvh"][:], w3["wvh"])
        nc.scalar.dma_start(wt["wvl"][:], w3["wvl"])
        nc.scalar.dma_start(wpth[:], wph3)
        nc.scalar.dma_start(wptl[:], wpl3)
        # rowsum column of v: ones * S_V (gpsimd; DVE is busy with psum moves)
        nc.gpsimd.tensor_copy(
            vtp[:, :, :, D:W65],
            on1t[:, None, :, None].broadcast_to((P, NKT, CO, 1)))

        # ---------- emission generators ----------
        def gen_qk(proj, g, tb):
            """q/k projection for one 128-wide j-slice, one 512-t block."""
            wh, wl = wt[f"w{proj}h"], wt[f"w{proj}l"]
            xh_t, xl_t = xts[tb]
            terms = ((xh_t, wh), (xh_t, wl), (xl_t, wh))
            dst = (qt if proj == "q" else kt)[g]
            gs = slice(g * P, (g + 1) * P)
            ps = bgp.tile([P, QB], F32, tag="bg", name=f"{proj}{g}t{tb}")
            # halves sequential: a start=True re-arms the bank zero-region
            for h in range(2):
                hs = slice(h * 256, (h + 1) * 256)
                for cp in range(NCP):
                    s = slice(2 * cp, 2 * cp + 2)
                    for ti, (mv, st) in enumerate(terms):
                        nc.tensor.matmul(
                            ps[:, hs], st[:, s, gs], mv[:, s, hs],
                            start=(cp == 0 and ti == 0),
                            stop=(cp == NCP - 1 and ti == 2),
                            perf_mode=DR)
                        yield 128
            nc.vector.tensor_copy(dst[:, tb * QB:(tb + 1) * QB], ps[:])
            yield 0

        def gen_v(tt, tb):
            """v projection (natural layout) for one 128-t tile."""
            wh, wl = wt["wvh"], wt["wvl"]
            xh_t, xl_t = xts[tb]
            terms = ((xh_t, wh), (xh_t, wl), (xl_t, wh))
            ki = tb * 4 + tt
            ts_ = slice(tt * P, (tt + 1) * P)
            ps = bgp.tile([P, QB], F32, tag="bg", name=f"v{ki}")
            for h in range(2):
                hs = slice(h * 256, (h + 1) * 256)
                for cp in range(NCP):
                    s = slice(2 * cp, 2 * cp + 2)
                    for ti, (mv, st) in enumerate(terms):
                        nc.tensor.matmul(
                            ps[:, hs], mv[:, s, ts_], st[:, s, hs],
                            start=(cp == 0 and ti == 0),
                            stop=(cp == NCP - 1 and ti == 2),
                            perf_mode=DR)
                        yield 128
            nc.vector.tensor_copy(
                vtp[:, ki, :, 0:D],
                ps[:].rearrange("p (h d) -> p h d", d=D))
            yield 0

        def gen_tr(g, qb):
            """transpose y_norm -> yT for one head-pair, one 512-t block."""
            psf = bgp.tile([P, QB], F32, tag="bg", name=f"tr{g}q{qb}")
            tp = psf[:].bitcast(BF16)[:, 0:QB].rearrange(
                "p (a b) -> p a b", a=4)
            for qt_ in range(4):
                nc.tensor.matmul(
                    tp[:, qt_, :],
                    ynm[g][:, qb * 4 + qt_, :, :].rearrange("p a b -> p (a b)"),
                    idtt[:], is_transpose=True)
                yield 128
            cs = slice(qb * QB, (qb + 1) * QB)
            nc.vector.tensor_copy(
                yts[g][:, cs].rearrange("p (a b) -> p a b", a=4), tp[:])
            yield 0
            nc.vector.tensor_copy(ytsh[:, g, cs], yts[g][:, cs])
            yield 0
            nc.vector.tensor_tensor(
                ytsl[:, g, cs], yts[g][:, cs], ytsh[:, g, cs],
                mybir.AluOpType.subtract)
            yield 0

        def gen_po(tt, mh, copy_on_scalar=False):
            """output projection for one [128 t, 512 m] tile + store."""
            po = bgp.tile([P, QB], F32, tag="bg", name=f"po{tt}m{mh}")
            ts_ = slice(tt * P, (tt + 1) * P)
            for h in range(2):
                hs = slice(mh * QB + h * 256, mh * QB + (h + 1) * 256)
                os_ = slice(h * 256, (h + 1) * 256)
                for ti, (ya, wa) in enumerate(
                        ((ytsh, wpth), (ytsh, wptl), (ytsl, wpth))):
                    for pr in range(2):
                        s = slice(2 * pr, 2 * pr + 2)
                        nc.tensor.matmul(
                            po[:, os_], ya[:, s, ts_], wa[:, s, hs],
                            start=(ti == 0 and pr == 0),
                            stop=(ti == 2 and pr == 1),
                            perf_mode=DR)
                        yield 128
            ob = outp.tile([P, QB], F32, tag="ob", name="ob")
            if copy_on_scalar:
                nc.scalar.activation(ob[:], po[:], AF.Copy, scale=OUT_SCALE)
            else:
                nc.vector.tensor_scalar_mul(ob[:], po[:], OUT_SCALE)
            nc.sync.dma_start(
                out[tt * P:(tt + 1) * P, mh * QB:(mh + 1) * QB], ob[:])
            yield 0

        def run(gen):
            for _ in gen:
                pass

        # ---------- attention block ----------
        def att_block(g, qb, bg, pre_av=None):
            q0 = qb * QB
            ks = list(range(qb * 4, qb * 4 + 4)) + list(range(0, qb * 4))
            yp = avp.tile([P, 4, 2, P], F32, tag="yp", name="yp")
            nc.vector.memset(yp[:, :, :, 0:W65], 0.0)
            barrier = [pre_av]

            def av(job):
                if barrier[0] is not None:
                    barrier[0]()
                    barrier[0] = None
                ki, d, ae = job
                qt0 = 0 if d < 0 else d // P
                for h2 in range(2):
                    h = 2 * g + h2
                    for qt_ in range(qt0, 4):
                        nc.tensor.matmul(
                            yp[:, qt_, h2, 0:W65],
                            ae[:, h2, qt_ * P:(qt_ + 1) * P],
                            vtp[:, ki, h, 0:W65],
                            start=False, stop=True, skip_group_check=True)

            pend = []
            for idx, ki in enumerate(ks):
                d = (ki - qb * 4) * P if ki >= qb * 4 else -1
                dq = d if d in (P, 2 * P) else (2 * P if d == 3 * P else 0)
                ap_t = attp.tile([P, 2, QB], F32, tag="att", name="att")
                for h2 in range(2):
                    rows = slice(h2 * D, h2 * D + D)
                    nc.tensor.matmul(
                        ap_t[:, h2, dq:QB],
                        kt[g][rows, ki * P:(ki + 1) * P],
                        qt[g][rows, q0 + dq:q0 + QB],
                        start=True, stop=True)
                ae = aep.tile([P, 2, QB], BF16, tag="ae", name="ae")
                e0 = max(d, 0)
                nc.scalar.activation(ae[:, :, e0:QB], ap_t[:, :, e0:QB],
                                     AF.Exp, scale=EXP_SCALE)
                if d >= 0:
                    for h2 in range(2):
                        nc.gpsimd.affine_select(
                            out=ae[:, h2, d:d + P],
                            in_=ae[:, h2, d:d + P],
                            compare_op=mybir.AluOpType.is_ge,
                            fill=0.0, base=0,
                            pattern=[[1, P]], channel_multiplier=-1)
                pend.append((ki, d, ae))
                if len(pend) > CFG["lag"]:
                    av(pend.pop(0))
                bg.drain_rows(CFG["quantum"])
            while pend:
                av(pend.pop(0))

            rc = rcp.tile([P, 4, 2], F32, tag="rc", name="rc")
            nc.vector.reciprocal_approx_fast(rc[:], yp[:, :, :, D])
            nc.vector.tensor_tensor(
                ynm[g][:, qb * 4:(qb + 1) * 4, :, :],
                yp[:, :, :, 0:D],
                rc[:, :, :, None].broadcast_to((P, 4, 2, D)), MUL)

        # ---------- fused schedule ----------
        bg = Bg()
        # head: only q/k for head-pair 0; v follows in bg (first-AV barrier)
        run(gen_qk("q", 0, 0))
        run(gen_qk("k", 0, 0))
        for tt in range(4):
            bg.add("v0", gen_v(tt, 0))
        for g in range(1, 4):
            bg.add(f"qk{g}t0", gen_qk("q", g, 0))
            bg.add(f"qk{g}t0", gen_qk("k", g, 0))

        for qb in range(NQB):
            tbn = qb + 1
            if tbn < NQB:
                xh_t = xtp.tile([P, CO, QB], F8, tag="xh", name=f"xh{tbn}")
                xl_t = xtp.tile([P, CO, QB], F8, tag="xl", name=f"xl{tbn}")
                nc.sync.dma_start(xh_t[:], xh3[:, :, tbn * QB:(tbn + 1) * QB])
                nc.gpsimd.dma_start(xl_t[:], xl3[:, :, tbn * QB:(tbn + 1) * QB])
                xts[tbn] = (xh_t, xl_t)
                for tt in range(4):
                    bg.add(f"v{tbn}", gen_v(tt, tbn))
                for g in range(2):
                    bg.add(f"qk{g}t{tbn}", gen_qk("q", g, tbn))
                    bg.add(f"qk{g}t{tbn}", gen_qk("k", g, tbn))
            if qb > 0:
                for g in range(4):
                    bg.add(f"tr{qb - 1}", gen_tr(g, qb - 1))
            if qb >= 2:
                for tt in range(4 * (qb - 2), 4 * (qb - 1)):
                    for mh in range(2):
                        bg.add(f"po{qb - 2}", gen_po(tt, mh))
            if qb == NQB - 1:
                # last qb gets extra filler: its predecessor's out-proj
                for tt in range(4 * (qb - 1), 4 * qb):
                    for mh in range(2):
                        bg.add(f"po{qb - 1}", gen_po(tt, mh))
            if tbn < NQB:
                # late j-slices feed the back half of this qb / early next qb
                for g in range(2, 4):
                    bg.add(f"qk{g}t{tbn}", gen_qk("q", g, tbn))
                    bg.add(f"qk{g}t{tbn}", gen_qk("k", g, tbn))
            for g in range(4):
                bg.drain_until(f"qk{g}t{qb}")
                pre = (lambda q_=qb: bg.drain_until(f"v{q_}")) if g == 0 else None
                att_block(g, qb, bg, pre_av=pre)
                if qb == NQB - 1:
                    bg.add("tr3", gen_tr(g, qb))

        bg.drain_all()
        for tt in range(4 * (NQB - 1), 4 * NQB):
            for mh in range(2):
                run(gen_po(tt, mh, copy_on_scalar=True))

    nc.finalize()
    return nc


def _prep_inputs(x, Wq, Wk, Wv, Wp):
    import ml_dtypes
    F8 = ml_dtypes.float8_e4m3
    BF = ml_dtypes.bfloat16
    f32 = np.float32

    def hilo(a):
        hi = np.ascontiguousarray(a).astype(F8)
        lo = (a - hi.astype(f32)).astype(F8)
        return hi, lo

    on1 = np.full((P, CO), S_V / Y_S, BF)
    idt = np.eye(P, dtype=BF)
    in_maps = []
    for c in range(NC):
        b, g2 = c // 2, c % 2
        j0 = g2 * 512
        xhc, xlc = hilo(x[b].T.astype(f32))
        wqh, wql = hilo((Wq[j0:j0 + 512] * (S_Q / 8.0)).T.astype(f32))
        wkh, wkl = hilo((Wk[j0:j0 + 512] * S_K).T.astype(f32))
        wvh, wvl = hilo((Wv[j0:j0 + 512] * S_V).T.astype(f32))
        wph_, wpl_ = hilo(np.ascontiguousarray(Wp[:, j0:j0 + 512].T * S_P)
                          .astype(f32))
        in_maps.append({
            "xh": xhc, "xl": xlc,
            "wqh": wqh, "wql": wql,
            "wkh": wkh, "wkl": wkl,
            "wvh": wvh, "wvl": wvl,
            "wph": wph_, "wpl": wpl_,
            "on1": on1, "idt": idt,
        })
    return in_maps


def kernel(x, Wq, Wk, Wv, Wp, _trace=False):
    from concourse.bass_utils import run_bass_kernel_spmd

    x = np.asarray(x); Wq = np.asarray(Wq); Wk = np.asarray(Wk)
    Wv = np.asarray(Wv); Wp = np.asarray(Wp)

    if "nc" not in _CACHE:
        _CACHE["nc"] = _build()
    nc = _CACHE["nc"]

    in_maps = _prep_inputs(x, Wq, Wk, Wv, Wp)
    res = run_bass_kernel_spmd(nc, in_maps, core_ids=list(range(NC)),
                               trace=_trace)
    outs = [r["out"] for r in res.results]
    full = np.empty((B, T, C), np.float32)
    for b in range(B):
        full[b] = outs[2 * b] + outs[2 * b + 1]
    if _trace:
        _CACHE["last_results"] = res
    return full


# revision 52
# speedup vs baseline: 1.0153x; 1.0017x over previous
"""Causal self-attention (B=4, T=2048, C=1024, H=16) on 8 TRN2 NeuronCores.

Sharding: core c -> batch b = c//2, head-group g2 = c%2 (8 heads, feature
columns j0 = g2*512 .. +512).  Each core:
  - QKV projections for its 512-wide slice (Megatron column-parallel),
  - causal attention for its 8 heads (softmax without max-subtraction:
    logits ~ N(0,1), folded 1/sqrt(hd) into Wq on host),
  - partial output projection y_half @ Wp[:, slice].T.
Host sums the two partials per batch.  No collectives.

v3 dataflow (per core):
  - QKV projections in fp8e4 DoubleRow (2 k-tiles / matmul, 0.5 cyc/row)
    with first-order error compensation: x and W are split hi/lo into two
    fp8 tensors on the host (per-tensor power-of-2 scales keep the lo parts
    out of the fp8 subnormal range); psum accumulates xh*wh + xh*wl + xl*wh.
  - QK^T per k-tile in fp32r (exp amplifies logit error; fp8 not safe).
  - exp on ScalarE with scale=2^-13 (undoes the host scales), bf16 out.
  - causal staircase zeroed by gpsimd affine_select (bf16).
  - AV with the attention tile as the *stationary* operand [128k x 128q]
    and [v | ones]-moving (65 columns out): rowsum lands in psum column 64,
    so normalization is a per-partition reciprocal + one broadcast multiply
    on DVE (no broadcast matmuls).  AV psums accumulate with start=False
    onto gpsimd-memset banks (8 interleaved groups share 2 banks; the HW
    2KB zero-region would corrupt interleaved start=True groups).
  - y transposed back to [i, t] via PE transpose (bf16, 2 heads / transpose),
    then the output projection contracts in bf16.
  - fused software-pipelined schedule: the attention phase is ACT(exp)-
    limited, so next-t-block projections and previous-block transposes/
    out-projections are drained into the attention blocks as background PE
    work between k-tiles (the `Bg` queue of emission generators).
"""
import numpy as np

B, T, C = 4, 2048, 1024
NC = 8
P = 128
CO = 8           # c-tiles of 128 (contraction for QKV)
NCP = 4          # co-pairs (DoubleRow contracts 2 c-tiles per matmul)
QB = 512         # t_q block
NQB = T // QB    # 4
NKT = T // P     # 16 k-tiles
D = 64           # head dim
W65 = 65         # [v | ones]

S_Q = 256.0      # host scale on Wq/8 (fp8 dynamic range)
S_K = 32.0       # host scale on Wk
S_V = 32.0       # host scale on Wv
Y_S = 8.0        # y_norm scale: ones = S_V/Y_S makes y_norm = Y_S * y
S_P = 32.0       # host scale on Wp (fp8 dynamic range)
EXP_SCALE = 1.0 / (S_Q * S_K)   # 2^-13, applied inside the exp activation
OUT_SCALE = 1.0 / (Y_S * S_P)   # folded into the final psum->sbuf copy

_CACHE = {}

# build-time tuning knobs (sweepable)
CFG = {"attp": 2, "aep": 13, "lag": 8, "quantum": 832}


class Bg:
    """Queue of emission generators drained between attention k-tiles.

    Each generator yields the matmul-row count it just emitted; items are
    labeled so attention blocks can force-drain their dependencies.
    """

    def __init__(self):
        self.items = []

    def add(self, label, gen):
        self.items.append((label, gen))

    def rows_left(self):
        return sum(1 for _ in self.items)  # item count proxy (unused)

    def drain_rows(self, target):
        done = 0
        while self.items and done < target:
            _, g = self.items[0]
            try:
                done += next(g)
            except StopIteration:
                self.items.pop(0)
        return done

    def drain_until(self, label):
        while any(l == label for l, _ in self.items):
            _, g = self.items[0]
            try:
                next(g)
            except StopIteration:
                self.items.pop(0)

    def drain_all(self):
        while self.items:
            _, g = self.items[0]
            try:
                next(g)
            except StopIteration:
                self.items.pop(0)


def _build():
    from contextlib import ExitStack
    import concourse.bass as bass
    import concourse.tile as tile
    from concourse import bacc, mybir

    F32 = mybir.dt.float32
    F32R = mybir.dt.float32r
    F8 = mybir.dt.float8e4
    BF16 = mybir.dt.bfloat16
    AF = mybir.ActivationFunctionType
    MUL = mybir.AluOpType.mult
    DR = mybir.MatmulPerfMode.DoubleRow

    nc = bacc.Bacc("TRN2", target_bir_lowering=False, debug=False,
                   dynamic_dma_scratch_size=2048)
    xh = nc.dram_tensor("xh", [C, T], F8, kind="ExternalInput").ap()
    xl = nc.dram_tensor("xl", [C, T], F8, kind="ExternalInput").ap()
    wts_d = {}
    for nm in ("wqh", "wql", "wkh", "wkl", "wvh", "wvl"):
        wts_d[nm] = nc.dram_tensor(nm, [C, 512], F8, kind="ExternalInput").ap()
    wph = nc.dram_tensor("wph", [512, C], F8, kind="ExternalInput").ap()
    wpl = nc.dram_tensor("wpl", [512, C], F8, kind="ExternalInput").ap()
    on1 = nc.dram_tensor("on1", [P, CO], BF16, kind="ExternalInput").ap()
    idt = nc.dram_tensor("idt", [P, P], BF16, kind="ExternalInput").ap()
    out = nc.dram_tensor("out", [T, C], F32, kind="ExternalOutput").ap()

    xh3 = xh.rearrange("(co ci) t -> ci co t", ci=P)     # [128, 8, 2048]
    xl3 = xl.rearrange("(co ci) t -> ci co t", ci=P)
    w3 = {nm: a.rearrange("(co ci) j -> ci co j", ci=P)
          for nm, a in wts_d.items()}                    # [128, 8, 512]
    wph3 = wph.rearrange("(go gi) m -> gi go m", gi=P)   # [128, 4, 1024]
    wpl3 = wpl.rearrange("(go gi) m -> gi go m", gi=P)

    with tile.TileContext(nc) as tc, ExitStack() as ctx:
        persist = ctx.enter_context(tc.tile_pool(name="persist", bufs=1))
        qt = [persist.tile([P, T], F32R, tag=f"qt{g}", name=f"qt{g}") for g in range(4)]
        kt = [persist.tile([P, T], F32R, tag=f"kt{g}", name=f"kt{g}") for g in range(4)]
        vtp = persist.tile([P, NKT, CO, W65], BF16, tag="vtp", name="vtp")
        # normalized y, qtile-major: [q-pos, qtile, h2, d] (contiguous
        # [128,128] per-qtile slice for the PE transpose)
        ynm = [persist.tile([P, NKT, 2, D], BF16, tag=f"ynm{g}", name=f"ynm{g}")
               for g in range(4)]
        yts = [persist.tile([P, T], BF16, tag=f"yts{g}", name=f"yts{g}")
               for g in range(4)]
        ytsh = persist.tile([P, 4, T], F8, tag="ytsh", name="ytsh")
        ytsl = persist.tile([P, 4, T], F8, tag="ytsl", name="ytsl")
        on1t = persist.tile([P, CO], BF16, tag="on1", name="on1")
        idtt = persist.tile([P, P], BF16, tag="idt", name="idt")
        wpth = persist.tile([P, 4, C], F8, tag="wpth", name="wpth")
        wptl = persist.tile([P, 4, C], F8, tag="wptl", name="wptl")

        xtp = ctx.enter_context(tc.tile_pool(name="xtp", bufs=2))
        wpool = ctx.enter_context(tc.tile_pool(name="wqkv", bufs=1))
        bgp = ctx.enter_context(
            tc.tile_pool(name="bgp", bufs=2, space="PSUM"))
        attp = ctx.enter_context(
            tc.tile_pool(name="attp", bufs=CFG["attp"], space="PSUM"))
        avp = ctx.enter_context(tc.tile_pool(name="avp", bufs=1, space="PSUM"))
        aep = ctx.enter_context(tc.tile_pool(name="aep", bufs=CFG["aep"]))
        rcp = ctx.enter_context(tc.tile_pool(name="rcp", bufs=2))
        outp = ctx.enter_context(tc.tile_pool(name="outp", bufs=4))

        wt = {}
        for nm in ("wqh", "wql", "wkh", "wkl", "wvh", "wvl"):
            wt[nm] = wpool.tile([P, CO, 512], F8, tag=nm, name=nm)

        # ---- input DMAs: hi parts on sync, lo parts on vector (parallel
        # queues halve the head's arrival ramp); wk on gpsimd, v/wp on scalar
        xts = {}
        xh0 = xtp.tile([P, CO, QB], F8, tag="xh", name="xh0")
        xl0 = xtp.tile([P, CO, QB], F8, tag="xl", name="xl0")
        xts[0] = (xh0, xl0)
        for cp in range(NCP):
            s = slice(2 * cp, 2 * cp + 2)
            nc.sync.dma_start(wt["wqh"][:, s], w3["wqh"][:, s])
            nc.sync.dma_start(xh0[:, s], xh3[:, s, 0:QB])
            nc.gpsimd.dma_start(wt["wql"][:, s], w3["wql"][:, s])
            nc.gpsimd.dma_start(xl0[:, s], xl3[:, s, 0:QB])
        nc.scalar.dma_start(wt["wkh"][:], w3["wkh"])
        nc.scalar.dma_start(wt["wkl"][:], w3["wkl"])
        nc.scalar.dma_start(on1t[:], on1)
        nc.scalar.dma_start(idtt[:], idt)
        nc.scalar.dma_start(wt["wvh"][:], w3["wvh"])
        nc.scalar.dma_start(wt["wvl"][:], w3["wvl"])
        nc.scalar.dma_start(wpth[:], wph3)
        nc.scalar.dma_start(wptl[:], wpl3)
        # rowsum column of v: ones * S_V (gpsimd; DVE is busy with psum moves)
        nc.gpsimd.tensor_copy(
            vtp[:, :, :, D:W65],
            on1t[:, None, :, None].broadcast_to((P, NKT, CO, 1)))

        # ---------- emission generators ----------
        def gen_qk(proj, g, tb):
            """q/k projection for one 128-wide j-slice, one 512-t block."""
            wh, wl = wt[f"w{proj}h"], wt[f"w{proj}l"]
            xh_t, xl_t = xts[tb]
            terms = ((xh_t, wh), (xh_t, wl), (xl_t, wh))
            dst = (qt if proj == "q" else kt)[g]
            gs = slice(g * P, (g + 1) * P)
            ps = bgp.tile([P, QB], F32, tag="bg", name=f"{proj}{g}t{tb}")
            # halves sequential: a start=True re-arms the bank zero-region
            for h in range(2):
                hs = slice(h * 256, (h + 1) * 256)
                for cp in range(NCP):
                    s = slice(2 * cp, 2 * cp + 2)
                    for ti, (mv, st) in enumerate(terms):
                        nc.tensor.matmul(
                            ps[:, hs], st[:, s, gs], mv[:, s, hs],
                            start=(cp == 0 and ti == 0),
                            stop=(cp == NCP - 1 and ti == 2),
                            perf_mode=DR)
                        yield 128
            nc.vector.tensor_copy(dst[:, tb * QB:(tb + 1) * QB], ps[:])
            yield 0

        def gen_v(tt, tb):
            """v projection (natural layout) for one 128-t tile."""
            wh, wl = wt["wvh"], wt["wvl"]
            xh_t, xl_t = xts[tb]
            terms = ((xh_t, wh), (xh_t, wl), (xl_t, wh))
            ki = tb * 4 + tt
            ts_ = slice(tt * P, (tt + 1) * P)
            ps = bgp.tile([P, QB], F32, tag="bg", name=f"v{ki}")
            for h in range(2):
                hs = slice(h * 256, (h + 1) * 256)
                for cp in range(NCP):
                    s = slice(2 * cp, 2 * cp + 2)
                    for ti, (mv, st) in enumerate(terms):
                        nc.tensor.matmul(
                            ps[:, hs], mv[:, s, ts_], st[:, s, hs],
                            start=(cp == 0 and ti == 0),
                            stop=(cp == NCP - 1 and ti == 2),
                            perf_mode=DR)
                        yield 128
            nc.vector.tensor_copy(
                vtp[:, ki, :, 0:D],
                ps[:].rearrange("p (h d) -> p h d", d=D))
            yield 0

        def gen_tr(g, qb):
            """transpose y_norm -> yT for one head-pair, one 512-t block."""
            psf = bgp.tile([P, QB], F32, tag="bg", name=f"tr{g}q{qb}")
            tp = psf[:].bitcast(BF16)[:, 0:QB].rearrange(
                "p (a b) -> p a b", a=4)
            for qt_ in range(4):
                nc.tensor.matmul(
                    tp[:, qt_, :],
                    ynm[g][:, qb * 4 + qt_, :, :].rearrange("p a b -> p (a b)"),
                    idtt[:], is_transpose=True)
                yield 128
            cs = slice(qb * QB, (qb + 1) * QB)
            nc.vector.tensor_copy(
                yts[g][:, cs].rearrange("p (a b) -> p a b", a=4), tp[:])
            yield 0
            nc.vector.tensor_copy(ytsh[:, g, cs], yts[g][:, cs])
            yield 0
            nc.vector.tensor_tensor(
                ytsl[:, g, cs], yts[g][:, cs], ytsh[:, g, cs],
                mybir.AluOpType.subtract)
            yield 0

        def gen_po(tt, mh, copy_on_scalar=False):
            """output projection for one [128 t, 512 m] tile + store."""
            po = bgp.tile([P, QB], F32, tag="bg", name=f"po{tt}m{mh}")
            ts_ = slice(tt * P, (tt + 1) * P)
            for h in range(2):
                hs = slice(mh * QB + h * 256, mh * QB + (h + 1) * 256)
                os_ = slice(h * 256, (h + 1) * 256)
                for ti, (ya, wa) in enumerate(
                        ((ytsh, wpth), (ytsh, wptl), (ytsl, wpth))):
                    for pr in range(2):
                        s = slice(2 * pr, 2 * pr + 2)
                        nc.tensor.matmul(
                            po[:, os_], ya[:, s, ts_], wa[:, s, hs],
                            start=(ti == 0 and pr == 0),
                            stop=(ti == 2 and pr == 1),
                            perf_mode=DR)
                        yield 128
            ob = outp.tile([P, QB], F32, tag="ob", name="ob")
            if copy_on_scalar:
                nc.scalar.activation(ob[:], po[:], AF.Copy, scale=OUT_SCALE)
            else:
                nc.vector.tensor_scalar_mul(ob[:], po[:], OUT_SCALE)
            nc.sync.dma_start(
                out[tt * P:(tt + 1) * P, mh * QB:(mh + 1) * QB], ob[:])
            yield 0

        def run(gen):
            for _ in gen:
                pass

        # ---------- attention block ----------
        def att_block(g, qb, bg, pre_av=None):
            q0 = qb * QB
            ks = list(range(qb * 4, qb * 4 + 4)) + list(range(0, qb * 4))
            yp = avp.tile([P, 4, 2, P], F32, tag="yp", name="yp")
            nc.vector.memset(yp[:, :, :, 0:W65], 0.0)
            barrier = [pre_av]

            def av(job):
                if barrier[0] is not None:
                    barrier[0]()
                    barrier[0] = None
                ki, d, ae = job
                qt0 = 0 if d < 0 else d // P
                for h2 in range(2):
                    h = 2 * g + h2
                    for qt_ in range(qt0, 4):
                        nc.tensor.matmul(
                            yp[:, qt_, h2, 0:W65],
                            ae[:, h2, qt_ * P:(qt_ + 1) * P],
                            vtp[:, ki, h, 0:W65],
                            start=False, stop=True, skip_group_check=True)

            pend = []
            for idx, ki in enumerate(ks):
                d = (ki - qb * 4) * P if ki >= qb * 4 else -1
                dq = d if d in (P, 2 * P) else (2 * P if d == 3 * P else 0)
                ap_t = attp.tile([P, 2, QB], F32, tag="att", name="att")
                for h2 in range(2):
                    rows = slice(h2 * D, h2 * D + D)
                    nc.tensor.matmul(
                        ap_t[:, h2, dq:QB],
                        kt[g][rows, ki * P:(ki + 1) * P],
                        qt[g][rows, q0 + dq:q0 + QB],
                        start=True, stop=True)
                ae = aep.tile([P, 2, QB], BF16, tag="ae", name="ae")
                e0 = max(d, 0)
                nc.scalar.activation(ae[:, :, e0:QB], ap_t[:, :, e0:QB],
                                     AF.Exp, scale=EXP_SCALE)
                if d >= 0:
                    for h2 in range(2):
                        nc.gpsimd.affine_select(
                            out=ae[:, h2, d:d + P],
                            in_=ae[:, h2, d:d + P],
                            compare_op=mybir.AluOpType.is_ge,
                            fill=0.0, base=0,
                            pattern=[[1, P]], channel_multiplier=-1)
                pend.append((ki, d, ae))
                if len(pend) > CFG["lag"]:
                    av(pend.pop(0))
                bg.drain_rows(CFG["quantum"])
            while pend:
                av(pend.pop(0))

            rc = rcp.tile([P, 4, 2], F32, tag="rc", name="rc")
            nc.vector.reciprocal_approx_fast(rc[:], yp[:, :, :, D])
            nc.vector.tensor_tensor(
                ynm[g][:, qb * 4:(qb + 1) * 4, :, :],
                yp[:, :, :, 0:D],
                rc[:, :, :, None].broadcast_to((P, 4, 2, D)), MUL)

        # ---------- fused schedule ----------
        bg = Bg()
        # head: only q/k for head-pair 0; v follows in bg (first-AV barrier)
        run(gen_qk("q", 0, 0))
        run(gen_qk("k", 0, 0))
        for tt in range(4):
            bg.add("v0", gen_v(tt, 0))
        for g in range(1, 4):
            bg.add(f"qk{g}t0", gen_qk("q", g, 0))
            bg.add(f"qk{g}t0", gen_qk("k", g, 0))

        for qb in range(NQB):
            tbn = qb + 1
            if tbn < NQB:
                xh_t = xtp.tile([P, CO, QB], F8, tag="xh", name=f"xh{tbn}")
                xl_t = xtp.tile([P, CO, QB], F8, tag="xl", name=f"xl{tbn}")
                nc.sync.dma_start(xh_t[:], xh3[:, :, tbn * QB:(tbn + 1) * QB])
                nc.gpsimd.dma_start(xl_t[:], xl3[:, :, tbn * QB:(tbn + 1) * QB])
                xts[tbn] = (xh_t, xl_t)
                for tt in range(4):
                    bg.add(f"v{tbn}", gen_v(tt, tbn))
                for g in range(2):
                    bg.add(f"qk{g}t{tbn}", gen_qk("q", g, tbn))
                    bg.add(f"qk{g}t{tbn}", gen_qk("k", g, tbn))
            if qb > 0:
                for g in range(4):
                    bg.add(f"tr{qb - 1}", gen_tr(g, qb - 1))
            if qb >= 2:
                for tt in range(4 * (qb - 2), 4 * (qb - 1)):
                    for mh in range(2):
                        bg.add(f"po{qb - 2}", gen_po(tt, mh))
            if qb == NQB - 1:
                # last qb gets extra filler: its predecessor's out-proj
                for tt in range(4 * (qb - 1), 4 * qb):
                    for mh in range(2):
                        bg.add(f"po{qb - 1}", gen_po(tt, mh))
            if tbn < NQB:
                # late j-slices feed the back half of this qb / early next qb
                for g in range(2, 4):
                    bg.add(f"qk{g}t{tbn}", gen_qk("q", g, tbn))
                    bg.add(f"qk{g}t{tbn}", gen_qk("k", g, tbn))
            for g in range(4):
                bg.drain_until(f"qk{g}t{qb}")
                pre = (lambda q_=qb: bg.drain_until(f"v{q_}")) if g == 0 else None
                att_block(g, qb, bg, pre_av=pre)
                if qb == NQB - 1:
                    bg.add("tr3", gen_tr(g, qb))

        bg.drain_all()
        for tt in range(4 * (NQB - 1), 4 * NQB):
            for mh in range(2):
                run(gen_po(tt, mh, copy_on_scalar=True))

    nc.finalize()
    return nc


def _prep_inputs(x, Wq, Wk, Wv, Wp):
    import ml_dtypes
    F8 = ml_dtypes.float8_e4m3
    BF = ml_dtypes.bfloat16
    f32 = np.float32

    def hilo(a):
        hi = np.ascontiguousarray(a).astype(F8)
        lo = (a - hi.astype(f32)).astype(F8)
        return hi, lo

    on1 = np.full((P, CO), S_V / Y_S, BF)
    idt = np.eye(P, dtype=BF)
    in_maps = []
    for c in range(NC):
        b, g2 = c // 2, c % 2
        j0 = g2 * 512
        xhc, xlc = hilo(x[b].T.astype(f32))
        wqh, wql = hilo((Wq[j0:j0 + 512] * (S_Q / 8.0)).T.astype(f32))
        wkh, wkl = hilo((Wk[j0:j0 + 512] * S_K).T.astype(f32))
        wvh, wvl = hilo((Wv[j0:j0 + 512] * S_V).T.astype(f32))
        wph_, wpl_ = hilo(np.ascontiguousarray(Wp[:, j0:j0 + 512].T * S_P)
                          .astype(f32))
        in_maps.append({
            "xh": xhc, "xl": xlc,
            "wqh": wqh, "wql": wql,
            "wkh": wkh, "wkl": wkl,
            "wvh": wvh, "wvl": wvl,
            "wph": wph_, "wpl": wpl_,
            "on1": on1, "idt": idt,
        })
    return in_maps


def kernel(x, Wq, Wk, Wv, Wp, _trace=False):
    from concourse.bass_utils import run_bass_kernel_spmd

    x = np.asarray(x); Wq = np.asarray(Wq); Wk = np.asarray(Wk)
    Wv = np.asarray(Wv); Wp = np.asarray(Wp)

    if "nc" not in _CACHE:
        _CACHE["nc"] = _build()
    nc = _CACHE["nc"]

    in_maps = _prep_inputs(x, Wq, Wk, Wv, Wp)
    res = run_bass_kernel_spmd(nc, in_maps, core_ids=list(range(NC)),
                               trace=_trace)
    outs = [r["out"] for r in res.results]
    full = np.empty((B, T, C), np.float32)
    for b in range(B):
        full[b] = outs[2 * b] + outs[2 * b + 1]
    if _trace:
        _CACHE["last_results"] = res
    return full


# revision 56
# speedup vs baseline: 1.0182x; 1.0029x over previous
"""Causal self-attention (B=4, T=2048, C=1024, H=16) on 8 TRN2 NeuronCores.

Sharding: core c -> batch b = c//2, head-group g2 = c%2 (8 heads, feature
columns j0 = g2*512 .. +512).  Each core:
  - QKV projections for its 512-wide slice (Megatron column-parallel),
  - causal attention for its 8 heads (softmax without max-subtraction:
    logits ~ N(0,1), folded 1/sqrt(hd) into Wq on host),
  - partial output projection y_half @ Wp[:, slice].T.
Host sums the two partials per batch.  No collectives.

v3 dataflow (per core):
  - QKV projections in fp8e4 DoubleRow (2 k-tiles / matmul, 0.5 cyc/row)
    with first-order error compensation: x and W are split hi/lo into two
    fp8 tensors on the host (per-tensor power-of-2 scales keep the lo parts
    out of the fp8 subnormal range); psum accumulates xh*wh + xh*wl + xl*wh.
  - QK^T per k-tile in fp32r (exp amplifies logit error; fp8 not safe).
  - exp on ScalarE with scale=2^-13 (undoes the host scales), bf16 out.
  - causal staircase zeroed by gpsimd affine_select (bf16).
  - AV with the attention tile as the *stationary* operand [128k x 128q]
    and [v | ones]-moving (65 columns out): rowsum lands in psum column 64,
    so normalization is a per-partition reciprocal + one broadcast multiply
    on DVE (no broadcast matmuls).  AV psums accumulate with start=False
    onto gpsimd-memset banks (8 interleaved groups share 2 banks; the HW
    2KB zero-region would corrupt interleaved start=True groups).
  - y transposed back to [i, t] via PE transpose (bf16, 2 heads / transpose),
    then the output projection contracts in bf16.
  - fused software-pipelined schedule: the attention phase is ACT(exp)-
    limited, so next-t-block projections and previous-block transposes/
    out-projections are drained into the attention blocks as background PE
    work between k-tiles (the `Bg` queue of emission generators).
"""
import numpy as np

B, T, C = 4, 2048, 1024
NC = 8
P = 128
CO = 8           # c-tiles of 128 (contraction for QKV)
NCP = 4          # co-pairs (DoubleRow contracts 2 c-tiles per matmul)
QB = 512         # t_q block
NQB = T // QB    # 4
NKT = T // P     # 16 k-tiles
D = 64           # head dim
W65 = 65         # [v | ones]

S_Q = 256.0      # host scale on Wq/8 (fp8 dynamic range)
S_K = 32.0       # host scale on Wk
S_V = 32.0       # host scale on Wv
Y_S = 8.0        # y_norm scale: ones = S_V/Y_S makes y_norm = Y_S * y
S_P = 32.0       # host scale on Wp (fp8 dynamic range)
EXP_SCALE = 1.0 / (S_Q * S_K)   # 2^-13, applied inside the exp activation
OUT_SCALE = 1.0 / (Y_S * S_P)   # folded into the final psum->sbuf copy

_CACHE = {}

# build-time tuning knobs (sweepable)
CFG = {"attp": 2, "aep": 13, "lag": 8, "quantum": 832}


class Bg:
    """Queue of emission generators drained between attention k-tiles.

    Each generator yields the matmul-row count it just emitted; items are
    labeled so attention blocks can force-drain their dependencies.
    """

    def __init__(self):
        self.items = []

    def add(self, label, gen):
        self.items.append((label, gen))

    def rows_left(self):
        return sum(1 for _ in self.items)  # item count proxy (unused)

    def drain_rows(self, target):
        done = 0
        while self.items and done < target:
            _, g = self.items[0]
            try:
                done += next(g)
            except StopIteration:
                self.items.pop(0)
        return done

    def drain_until(self, label):
        while any(l == label for l, _ in self.items):
            _, g = self.items[0]
            try:
                next(g)
            except StopIteration:
                self.items.pop(0)

    def drain_all(self):
        while self.items:
            _, g = self.items[0]
            try:
                next(g)
            except StopIteration:
                self.items.pop(0)


def _build():
    from contextlib import ExitStack
    import concourse.bass as bass
    import concourse.tile as tile
    from concourse import bacc, mybir

    F32 = mybir.dt.float32
    F32R = mybir.dt.float32r
    F8 = mybir.dt.float8e4
    BF16 = mybir.dt.bfloat16
    AF = mybir.ActivationFunctionType
    MUL = mybir.AluOpType.mult
    DR = mybir.MatmulPerfMode.DoubleRow

    nc = bacc.Bacc("TRN2", target_bir_lowering=False, debug=False,
                   dynamic_dma_scratch_size=2048)
    xh = nc.dram_tensor("xh", [C, T], F8, kind="ExternalInput").ap()
    xl = nc.dram_tensor("xl", [C, T], F8, kind="ExternalInput").ap()
    wts_d = {}
    for nm in ("wqh", "wql", "wkh", "wkl", "wvh", "wvl"):
        wts_d[nm] = nc.dram_tensor(nm, [C, 512], F8, kind="ExternalInput").ap()
    wph = nc.dram_tensor("wph", [512, C], F8, kind="ExternalInput").ap()
    wpl = nc.dram_tensor("wpl", [512, C], F8, kind="ExternalInput").ap()
    on1 = nc.dram_tensor("on1", [P, CO], BF16, kind="ExternalInput").ap()
    idt = nc.dram_tensor("idt", [P, P], BF16, kind="ExternalInput").ap()
    out = nc.dram_tensor("out", [T, C], F32, kind="ExternalOutput").ap()

    xh3 = xh.rearrange("(co ci) t -> ci co t", ci=P)     # [128, 8, 2048]
    xl3 = xl.rearrange("(co ci) t -> ci co t", ci=P)
    w3 = {nm: a.rearrange("(co ci) j -> ci co j", ci=P)
          for nm, a in wts_d.items()}                    # [128, 8, 512]
    wph3 = wph.rearrange("(go gi) m -> gi go m", gi=P)   # [128, 4, 1024]
    wpl3 = wpl.rearrange("(go gi) m -> gi go m", gi=P)

    with tile.TileContext(nc) as tc, ExitStack() as ctx:
        persist = ctx.enter_context(tc.tile_pool(name="persist", bufs=1))
        qt = [persist.tile([P, T], F32R, tag=f"qt{g}", name=f"qt{g}") for g in range(4)]
        kt = [persist.tile([P, T], F32R, tag=f"kt{g}", name=f"kt{g}") for g in range(4)]
        vtp = persist.tile([P, NKT, CO, W65], BF16, tag="vtp", name="vtp")
        # normalized y, qtile-major: [q-pos, qtile, h2, d] (contiguous
        # [128,128] per-qtile slice for the PE transpose)
        ynm = [persist.tile([P, NKT, 2, D], BF16, tag=f"ynm{g}", name=f"ynm{g}")
               for g in range(4)]
        yts = [persist.tile([P, T], BF16, tag=f"yts{g}", name=f"yts{g}")
               for g in range(4)]
        ytsh = persist.tile([P, 4, T], F8, tag="ytsh", name="ytsh")
        ytsl = persist.tile([P, 4, T], F8, tag="ytsl", name="ytsl")
        on1t = persist.tile([P, CO], BF16, tag="on1", name="on1")
        idtt = persist.tile([P, P], BF16, tag="idt", name="idt")
        wpth = persist.tile([P, 4, C], F8, tag="wpth", name="wpth")
        wptl = persist.tile([P, 4, C], F8, tag="wptl", name="wptl")

        xtp = ctx.enter_context(tc.tile_pool(name="xtp", bufs=2))
        wpool = ctx.enter_context(tc.tile_pool(name="wqkv", bufs=1))
        bgp = ctx.enter_context(
            tc.tile_pool(name="bgp", bufs=2, space="PSUM"))
        attp = ctx.enter_context(
            tc.tile_pool(name="attp", bufs=CFG["attp"], space="PSUM"))
        avp = ctx.enter_context(tc.tile_pool(name="avp", bufs=1, space="PSUM"))
        aep = ctx.enter_context(tc.tile_pool(name="aep", bufs=CFG["aep"]))
        rcp = ctx.enter_context(tc.tile_pool(name="rcp", bufs=2))
        outp = ctx.enter_context(tc.tile_pool(name="outp", bufs=8))

        wt = {}
        for nm in ("wqh", "wql", "wkh", "wkl", "wvh", "wvl"):
            wt[nm] = wpool.tile([P, CO, 512], F8, tag=nm, name=nm)

        # ---- input DMAs: hi parts on sync, lo parts on vector (parallel
        # queues halve the head's arrival ramp); wk on gpsimd, v/wp on scalar
        xts = {}
        xh0 = xtp.tile([P, CO, QB], F8, tag="xh", name="xh0")
        xl0 = xtp.tile([P, CO, QB], F8, tag="xl", name="xl0")
        xts[0] = (xh0, xl0)
        for cp in range(NCP):
            s = slice(2 * cp, 2 * cp + 2)
            nc.sync.dma_start(wt["wqh"][:, s], w3["wqh"][:, s])
            nc.sync.dma_start(xh0[:, s], xh3[:, s, 0:QB])
            nc.gpsimd.dma_start(wt["wql"][:, s], w3["wql"][:, s])
            nc.gpsimd.dma_start(xl0[:, s], xl3[:, s, 0:QB])
        nc.scalar.dma_start(wt["wkh"][:], w3["wkh"])
        nc.scalar.dma_start(wt["wkl"][:], w3["wkl"])
        nc.scalar.dma_start(on1t[:], on1)
        nc.scalar.dma_start(idtt[:], idt)
        nc.scalar.dma_start(wt["wvh"][:], w3["wvh"])
        nc.scalar.dma_start(wt["wvl"][:], w3["wvl"])
        nc.scalar.dma_start(wpth[:], wph3)
        nc.scalar.dma_start(wptl[:], wpl3)
        # rowsum column of v: ones * S_V (gpsimd; DVE is busy with psum moves)
        nc.gpsimd.tensor_copy(
            vtp[:, :, :, D:W65],
            on1t[:, None, :, None].broadcast_to((P, NKT, CO, 1)))

        # ---------- emission generators ----------
        def gen_qk(proj, g, tb):
            """q/k projection for one 128-wide j-slice, one 512-t block."""
            wh, wl = wt[f"w{proj}h"], wt[f"w{proj}l"]
            xh_t, xl_t = xts[tb]
            terms = ((xh_t, wh), (xh_t, wl), (xl_t, wh))
            dst = (qt if proj == "q" else kt)[g]
            gs = slice(g * P, (g + 1) * P)
            ps = bgp.tile([P, QB], F32, tag="bg", name=f"{proj}{g}t{tb}")
            # halves sequential: a start=True re-arms the bank zero-region
            for h in range(2):
                hs = slice(h * 256, (h + 1) * 256)
                for cp in range(NCP):
                    s = slice(2 * cp, 2 * cp + 2)
                    for ti, (mv, st) in enumerate(terms):
                        nc.tensor.matmul(
                            ps[:, hs], st[:, s, gs], mv[:, s, hs],
                            start=(cp == 0 and ti == 0),
                            stop=(cp == NCP - 1 and ti == 2),
                            perf_mode=DR)
                        yield 128
            nc.vector.tensor_copy(dst[:, tb * QB:(tb + 1) * QB], ps[:])
            yield 0

        def gen_v(tt, tb):
            """v projection (natural layout) for one 128-t tile."""
            wh, wl = wt["wvh"], wt["wvl"]
            xh_t, xl_t = xts[tb]
            terms = ((xh_t, wh), (xh_t, wl), (xl_t, wh))
            ki = tb * 4 + tt
            ts_ = slice(tt * P, (tt + 1) * P)
            ps = bgp.tile([P, QB], F32, tag="bg", name=f"v{ki}")
            for h in range(2):
                hs = slice(h * 256, (h + 1) * 256)
                for cp in range(NCP):
                    s = slice(2 * cp, 2 * cp + 2)
                    for ti, (mv, st) in enumerate(terms):
                        nc.tensor.matmul(
                            ps[:, hs], mv[:, s, ts_], st[:, s, hs],
                            start=(cp == 0 and ti == 0),
                            stop=(cp == NCP - 1 and ti == 2),
                            perf_mode=DR)
                        yield 128
            nc.vector.tensor_copy(
                vtp[:, ki, :, 0:D],
                ps[:].rearrange("p (h d) -> p h d", d=D))
            yield 0

        def gen_tr(g, qb):
            """transpose y_norm -> yT for one head-pair, one 512-t block."""
            psf = bgp.tile([P, QB], F32, tag="bg", name=f"tr{g}q{qb}")
            tp = psf[:].bitcast(BF16)[:, 0:QB].rearrange(
                "p (a b) -> p a b", a=4)
            for qt_ in range(4):
                nc.tensor.matmul(
                    tp[:, qt_, :],
                    ynm[g][:, qb * 4 + qt_, :, :].rearrange("p a b -> p (a b)"),
                    idtt[:], is_transpose=True)
                yield 128
            cs = slice(qb * QB, (qb + 1) * QB)
            nc.vector.tensor_copy(
                yts[g][:, cs].rearrange("p (a b) -> p a b", a=4), tp[:])
            yield 0
            nc.vector.tensor_copy(ytsh[:, g, cs], yts[g][:, cs])
            yield 0
            nc.vector.tensor_tensor(
                ytsl[:, g, cs], yts[g][:, cs], ytsh[:, g, cs],
                mybir.AluOpType.subtract)
            yield 0

        def gen_po(tt, mh, copy_on_scalar=False):
            """output projection for one [128 t, 512 m] tile + store."""
            po = bgp.tile([P, QB], F32, tag="bg", name=f"po{tt}m{mh}")
            ts_ = slice(tt * P, (tt + 1) * P)
            for h in range(2):
                hs = slice(mh * QB + h * 256, mh * QB + (h + 1) * 256)
                os_ = slice(h * 256, (h + 1) * 256)
                for ti, (ya, wa) in enumerate(
                        ((ytsh, wpth), (ytsh, wptl), (ytsl, wpth))):
                    for pr in range(2):
                        s = slice(2 * pr, 2 * pr + 2)
                        nc.tensor.matmul(
                            po[:, os_], ya[:, s, ts_], wa[:, s, hs],
                            start=(ti == 0 and pr == 0),
                            stop=(ti == 2 and pr == 1),
                            perf_mode=DR)
                        yield 128
            ob = outp.tile([P, QB], F32, tag="ob", name="ob")
            if copy_on_scalar:
                nc.scalar.activation(ob[:], po[:], AF.Copy, scale=OUT_SCALE)
            else:
                nc.vector.tensor_scalar_mul(ob[:], po[:], OUT_SCALE)
            nc.sync.dma_start(
                out[tt * P:(tt + 1) * P, mh * QB:(mh + 1) * QB], ob[:])
            yield 0

        def run(gen):
            for _ in gen:
                pass

        # ---------- attention block ----------
        def att_block(g, qb, bg, pre_av=None):
            q0 = qb * QB
            ks = list(range(qb * 4, qb * 4 + 4)) + list(range(0, qb * 4))
            yp = avp.tile([P, 4, 2, P], F32, tag="yp", name="yp")
            nc.vector.memset(yp[:, :, :, 0:W65], 0.0)
            barrier = [pre_av]

            def av(job):
                if barrier[0] is not None:
                    barrier[0]()
                    barrier[0] = None
                ki, d, ae = job
                qt0 = 0 if d < 0 else d // P
                for h2 in range(2):
                    h = 2 * g + h2
                    for qt_ in range(qt0, 4):
                        nc.tensor.matmul(
                            yp[:, qt_, h2, 0:W65],
                            ae[:, h2, qt_ * P:(qt_ + 1) * P],
                            vtp[:, ki, h, 0:W65],
                            start=False, stop=True, skip_group_check=True)

            pend = []
            for idx, ki in enumerate(ks):
                d = (ki - qb * 4) * P if ki >= qb * 4 else -1
                dq = d if d in (P, 2 * P) else (2 * P if d == 3 * P else 0)
                ap_t = attp.tile([P, 2, QB], F32, tag="att", name="att")
                for h2 in range(2):
                    rows = slice(h2 * D, h2 * D + D)
                    nc.tensor.matmul(
                        ap_t[:, h2, dq:QB],
                        kt[g][rows, ki * P:(ki + 1) * P],
                        qt[g][rows, q0 + dq:q0 + QB],
                        start=True, stop=True)
                ae = aep.tile([P, 2, QB], BF16, tag="ae", name="ae")
                e0 = max(d, 0)
                nc.scalar.activation(ae[:, :, e0:QB], ap_t[:, :, e0:QB],
                                     AF.Exp, scale=EXP_SCALE)
                if d >= 0:
                    for h2 in range(2):
                        nc.gpsimd.affine_select(
                            out=ae[:, h2, d:d + P],
                            in_=ae[:, h2, d:d + P],
                            compare_op=mybir.AluOpType.is_ge,
                            fill=0.0, base=0,
                            pattern=[[1, P]], channel_multiplier=-1)
                pend.append((ki, d, ae))
                if len(pend) > CFG["lag"]:
                    av(pend.pop(0))
                bg.drain_rows(CFG["quantum"])
            while pend:
                av(pend.pop(0))

            rc = rcp.tile([P, 4, 2], F32, tag="rc", name="rc")
            nc.vector.reciprocal_approx_fast(rc[:], yp[:, :, :, D])
            nc.vector.tensor_tensor(
                ynm[g][:, qb * 4:(qb + 1) * 4, :, :],
                yp[:, :, :, 0:D],
                rc[:, :, :, None].broadcast_to((P, 4, 2, D)), MUL)

        # ---------- fused schedule ----------
        bg = Bg()
        # head: only q/k for head-pair 0; v follows in bg (first-AV barrier)
        run(gen_qk("q", 0, 0))
        run(gen_qk("k", 0, 0))
        for tt in range(4):
            bg.add("v0", gen_v(tt, 0))
        for g in range(1, 4):
            bg.add(f"qk{g}t0", gen_qk("q", g, 0))
            bg.add(f"qk{g}t0", gen_qk("k", g, 0))

        for qb in range(NQB):
            tbn = qb + 1
            if tbn < NQB:
                xh_t = xtp.tile([P, CO, QB], F8, tag="xh", name=f"xh{tbn}")
                xl_t = xtp.tile([P, CO, QB], F8, tag="xl", name=f"xl{tbn}")
                nc.sync.dma_start(xh_t[:], xh3[:, :, tbn * QB:(tbn + 1) * QB])
                nc.gpsimd.dma_start(xl_t[:], xl3[:, :, tbn * QB:(tbn + 1) * QB])
                xts[tbn] = (xh_t, xl_t)
                for tt in range(4):
                    bg.add(f"v{tbn}", gen_v(tt, tbn))
                for g in range(2):
                    bg.add(f"qk{g}t{tbn}", gen_qk("q", g, tbn))
                    bg.add(f"qk{g}t{tbn}", gen_qk("k", g, tbn))
            if qb > 0:
                for g in range(4):
                    bg.add(f"tr{qb - 1}", gen_tr(g, qb - 1))
            if qb >= 2:
                for tt in range(4 * (qb - 2), 4 * (qb - 1)):
                    for mh in range(2):
                        bg.add(f"po{qb - 2}", gen_po(tt, mh))
            if qb == NQB - 1:
                # last qb gets extra filler: its predecessor's out-proj
                for tt in range(4 * (qb - 1), 4 * qb):
                    for mh in range(2):
                        bg.add(f"po{qb - 1}", gen_po(tt, mh))
            if tbn < NQB:
                # late j-slices feed the back half of this qb / early next qb
                for g in range(2, 4):
                    bg.add(f"qk{g}t{tbn}", gen_qk("q", g, tbn))
                    bg.add(f"qk{g}t{tbn}", gen_qk("k", g, tbn))
            for g in range(4):
                bg.drain_until(f"qk{g}t{qb}")
                pre = (lambda q_=qb: bg.drain_until(f"v{q_}")) if g == 0 else None
                att_block(g, qb, bg, pre_av=pre)
                if qb == NQB - 1:
                    bg.add("tr3", gen_tr(g, qb))

        bg.drain_all()
        for tt in range(4 * (NQB - 1), 4 * NQB):
            for mh in range(2):
                run(gen_po(tt, mh, copy_on_scalar=True))

    nc.finalize()
    return nc


def _prep_inputs(x, Wq, Wk, Wv, Wp):
    import ml_dtypes
    F8 = ml_dtypes.float8_e4m3
    BF = ml_dtypes.bfloat16
    f32 = np.float32

    def hilo(a):
        hi = np.ascontiguousarray(a).astype(F8)
        lo = (a - hi.astype(f32)).astype(F8)
        return hi, lo

    on1 = np.full((P, CO), S_V / Y_S, BF)
    idt = np.eye(P, dtype=BF)
    in_maps = []
    for c in range(NC):
        b, g2 = c // 2, c % 2
        j0 = g2 * 512
        xhc, xlc = hilo(x[b].T.astype(f32))
        wqh, wql = hilo((Wq[j0:j0 + 512] * (S_Q / 8.0)).T.astype(f32))
        wkh, wkl = hilo((Wk[j0:j0 + 512] * S_K).T.astype(f32))
        wvh, wvl = hilo((Wv[j0:j0 + 512] * S_V).T.astype(f32))
        wph_, wpl_ = hilo(np.ascontiguousarray(Wp[:, j0:j0 + 512].T * S_P)
                          .astype(f32))
        in_maps.append({
            "xh": xhc, "xl": xlc,
            "wqh": wqh, "wql": wql,
            "wkh": wkh, "wkl": wkl,
            "wvh": wvh, "wvl": wvl,
            "wph": wph_, "wpl": wpl_,
            "on1": on1, "idt": idt,
        })
    return in_maps


def kernel(x, Wq, Wk, Wv, Wp, _trace=False):
    from concourse.bass_utils import run_bass_kernel_spmd

    x = np.asarray(x); Wq = np.asarray(Wq); Wk = np.asarray(Wk)
    Wv = np.asarray(Wv); Wp = np.asarray(Wp)

    if "nc" not in _CACHE:
        _CACHE["nc"] = _build()
    nc = _CACHE["nc"]

    in_maps = _prep_inputs(x, Wq, Wk, Wv, Wp)
    res = run_bass_kernel_spmd(nc, in_maps, core_ids=list(range(NC)),
                               trace=_trace)
    outs = [r["out"] for r in res.results]
    full = np.empty((B, T, C), np.float32)
    for b in range(B):
        full[b] = outs[2 * b] + outs[2 * b + 1]
    if _trace:
        _CACHE["last_results"] = res
    return full
